# revision 1
# baseline (speedup 1.0000x reference)
"""CoAtNet relative attention kernel for Trainium2 (Bass/Tile), 8 NeuronCores.

Problem (per full input):
  x [16, 256, 32, 32] f32; Wq/Wk/Wv [256, 256]; Wo [256, 256]; bo [256];
  rel_bias [8, 3969]; rel_idx [1024, 1024] int32 (static pattern).
  out[b] = softmax(q k^T / sqrt(d) + bias) v  projected back, heads=8, d=32.

Sharding: data-parallel over batch — each of the 8 cores handles 2 batches
with identical programs (SPMD, no collectives).

Key structural facts used:
  * rel_idx[p, q] == (q - p) + 1056 exactly (the reference's quirky *W stride
    collapses the 2D relative index to 1D Toeplitz).  So the [1024, 1024]
    bias matrix per head is bias[p, q] = rel_bias[h, q - p + 1056] and any
    [128, width] tile of it (keys on partitions) is a contiguous slice of a
    small "sheared" tile  G[h, i, j'] = rel_bias[h, 1952 + i - j']  of shape
    [128, 1920].  No gather on device at all.  The bias is applied as
    exp(S+B) = exp(S) * exp(B) with exp(B) precomputed, so the application
    is a bf16 2x-mode multiply instead of an fp32 1x add.
  * Everything is computed in "transposed" layout so no transposes are ever
    needed: x arrives as [c, n] per batch; Q^T/K^T = W @ x are [d_all, n];
    scores are built as S^T [keys, queries]; P@V uses lhsT = [V | 1]
    directly (the ones column emits the softmax denominator as a 33rd
    output row, one accumulation group per PSUM bank); and the final
    projection produces out^T [c, n], exactly the output memory layout.
  * Stage B processes 4 heads at once with row-tiled (tile_position)
    concurrent K=32 matmuls so the PE array stays dense/warm, and exp runs
    as one 2048-wide ACTIVATE per strip.
"""

import numpy as np
from contextlib import ExitStack

import concourse.bass as bass
import concourse.bacc as bacc
import concourse.mybir as mybir
import concourse.tile as tile
from concourse import bass_utils
from concourse._compat import with_exitstack

HEADS = 8
D = 32  # head dim
C = 256  # channels = heads * D
N = 1024  # tokens = 32 * 32
B_LOC = 2  # batches per core
N_CORES = 8
SCALE = D ** -0.5
GW = 1920  # sheared bias tile width
G0 = 1952  # G[h, i, j'] = rel_bias[h, G0 + i - j']

F32 = mybir.dt.float32
BF16 = mybir.dt.bfloat16
AF = mybir.ActivationFunctionType


@with_exitstack
def _emit(ctx: ExitStack, tc: tile.TileContext, io: dict):
    nc = tc.nc
    x, wqT, wkT, wvT, woT, bo, eb, out = (
        io[k] for k in ("x", "wqT", "wkT", "wvT", "woT", "bo", "eb", "out")
    )

    persist = ctx.enter_context(tc.tile_pool(name="persist", bufs=1))
    stexp_pool = ctx.enter_context(tc.tile_pool(name="stexp", bufs=3))
    small = ctx.enter_context(tc.tile_pool(name="small", bufs=2))
    outp = ctx.enter_context(tc.tile_pool(name="outp", bufs=4))
    dram_pool = ctx.enter_context(tc.tile_pool(name="dram", bufs=2, space="DRAM"))
    # PSUM: st [128, 2048] (4 banks) + ot [128, 1024] (2) + den [128, 1024]
    # (2) = all 8.  The projection stages ping-pong st/ot pools for overlap.
    ps_st = ctx.enter_context(tc.tile_pool(name="ps_st", bufs=1, space="PSUM"))
    ps_ot = ctx.enter_context(tc.tile_pool(name="ps_ot", bufs=2, space="PSUM"))
    ps_den = ctx.enter_context(tc.tile_pool(name="ps_den", bufs=2, space="PSUM"))

    def proj_psum(i):
        if i % 2 == 0:
            return ps_st.tile([128, 512], F32, tag="st", name="st_ps")
        return ps_ot.tile([128, 512], F32, tag="otden", name="ot_ps")

    # ---------- constants / weights / inputs ----------
    wq_sb, wk_sb, wv_sb, wo_sb = [], [], [], []
    for cc in range(2):
        for lst, src, nm in (
            (wq_sb, wqT, "wq"),
            (wk_sb, wkT, "wk"),
            (wv_sb, wvT, "wv"),
            (wo_sb, woT, "wo"),
        ):
            t = persist.tile([128, C], BF16, tag=f"{nm}{cc}", name=f"{nm}{cc}")
            nc.sync.dma_start(out=t[:], in_=src[128 * cc : 128 * (cc + 1), :])
            lst.append(t)
    bo_sb = []
    for cc in range(2):
        t = persist.tile([128, 1], F32, tag=f"bo{cc}", name=f"bo{cc}")
        nc.sync.dma_start(out=t[:], in_=bo[128 * cc : 128 * (cc + 1), :])
        bo_sb.append(t)
    ones32_sb = persist.tile([128, 32], BF16, tag="ones32", name="ones32")
    nc.vector.memset(ones32_sb[:], 1.0)

    x_sb = [[persist.tile([128, N], BF16, tag=f"x{b}_{cc}", name=f"x{b}_{cc}") for cc in range(2)] for b in range(B_LOC)]
    for b in range(B_LOC):
        for cc in range(2):
            nc.sync.dma_start(out=x_sb[b][cc][:], in_=x[b, 128 * cc : 128 * (cc + 1), :])
    # all 8 heads' exp-of-bias sheared tiles, side by side (needed only by
    # stage B, so loaded after the projection inputs)
    eb_sb = persist.tile([128, HEADS * GW], BF16, tag="eb", name="eb_sb")
    for h in range(HEADS):
        nc.sync.dma_start(out=eb_sb[:, GW * h : GW * (h + 1)], in_=eb[h])

    # ---------- stage A: projections ----------
    # qT/kT: [o, n] (o = h*32 + d), computed as (W^T)^T @ x = W @ x.
    qT_sb = [[persist.tile([128, N], BF16, tag=f"qT{b}_{oc}", name=f"qT{b}_{oc}") for oc in range(2)] for b in range(B_LOC)]
    kT_sb = [[persist.tile([128, N], BF16, tag=f"kT{b}_{oc}", name=f"kT{b}_{oc}") for oc in range(2)] for b in range(B_LOC)]
    # v: natural [n, o] layout, 8 row tiles of 128 tokens, with a ones column
    # appended per head (33 cols/head); the P@V matmul then emits the softmax
    # denominator as a 33rd output row in the same accumulation group.
    v_sb = [[persist.tile([128, 33 * HEADS], BF16, tag=f"v{b}_{nt}", name=f"v{b}_{nt}") for nt in range(8)] for b in range(B_LOC)]

    def emit_qk_group(b, oc, nc2, w_sb, dst, pool_tile):
        for cc in range(2):
            nc.tensor.matmul(
                pool_tile[:, 0:512],
                lhsT=w_sb[cc][:, 128 * oc : 128 * (oc + 1)],
                rhs=x_sb[b][cc][:, 512 * nc2 : 512 * (nc2 + 1)],
                start=(cc == 0),
                stop=(cc == 1),
            )
        nc.vector.tensor_copy(
            out=dst[b][oc][:, 512 * nc2 : 512 * (nc2 + 1)], in_=pool_tile[:, 0:512]
        )

    def emit_v_group(b, nt, pool_tile):
        for cc in range(2):
            nc.tensor.matmul(
                pool_tile[:, 0:C],
                lhsT=x_sb[b][cc][:, 128 * nt : 128 * (nt + 1)],
                rhs=wv_sb[cc][:],
                start=(cc == 0),
                stop=(cc == 1),
            )
        v33 = v_sb[b][nt][:].rearrange("p (h w) -> p h w", w=33)
        nc.vector.tensor_copy(
            out=v33[:, :, 0:32], in_=pool_tile[:, 0:C].rearrange("p (h w) -> p h w", w=32)
        )
        nc.vector.memset(v33[:, :, 32:33], 1.0)

    pi = 0
    for oc in range(2):
        for nc2 in range(2):
            for w_sb, dst in ((wq_sb, qT_sb), (wk_sb, kT_sb)):
                emit_qk_group(0, oc, nc2, w_sb, dst, proj_psum(pi))
                pi += 1
    for nt in range(8):
        emit_v_group(0, nt, proj_psum(pi))
        pi += 1
    # batch 1's projections are dripped into batch 0's attention loop (they
    # fill PE idle slivers there, and ACT starts ~12us earlier)
    deferred_proj = []
    for oc in range(2):
        for nc2 in range(2):
            for w_sb, dst in ((wq_sb, qT_sb), (wk_sb, kT_sb)):
                deferred_proj.append(
                    (lambda oc=oc, nc2=nc2, w_sb=w_sb, dst=dst: emit_qk_group(
                        1, oc, nc2, w_sb, dst,
                        ps_den.tile([128, 512], F32, tag="den", name="den_ps")))
                )
    for nt in range(8):
        deferred_proj.append(
            (lambda nt=nt: emit_v_group(
                1, nt, ps_den.tile([128, 512], F32, tag="den", name="den_ps")))
        )

    # ---------- stage B: attention, 4 heads (one quad) at a time ----------
    # out-projection rhs: chunk 0 = heads 0..3, chunk 1 = heads 4..7.
    otn_sb = [[persist.tile([128, N], BF16, tag=f"otn{b}_{ch}", name=f"otn{b}_{ch}") for ch in range(2)] for b in range(B_LOC)]

    # Software pipeline: PV matmuls lag one strip behind ST in the PE queue
    # (a stalled PV never head-of-line blocks ready ST work), and the previous
    # iteration's normalization chain (which has DMA bounces between DVE ops)
    # is drip-fed where its inputs are already available.
    pending_pv = None
    norm_parts = []

    def _emit_pv(args):
        ot_, den_, b_, kt_, se_, quad_, qi_, first, last = args
        for h2 in range(4):
            nc.tensor.matmul(
                ot_[32 * h2 : 32 * (h2 + 1), :],
                lhsT=v_sb[b_][kt_][:, 33 * (4 * quad_ + h2) : 33 * (4 * quad_ + h2) + 32],
                rhs=se_[:, 512 * h2 : 512 * (h2 + 1)],
                start=first,
                stop=last,
                tile_position=(0, 32 * h2),
                skip_group_check=True,
            )
        for h2 in range(4):
            nc.tensor.matmul(
                den_[32 * h2 : 32 * (h2 + 1), :],
                lhsT=ones32_sb[:],
                rhs=se_[:, 512 * h2 : 512 * (h2 + 1)],
                start=first,
                stop=last,
                tile_position=(0, 32 * h2),
                skip_group_check=True,
            )

    def _make_norm(ot_, den_, quad_, b_, qi_):
        # Normalize O^T by the softmax denominators (row 32 of each bank).
        # Reciprocal is iterative (8 cyc/elem/lane): reshape the [1, 2048]
        # denominator row to [32, 64] via a DRAM bounce to use 32 lanes, then
        # bounce again to broadcast the reciprocals across 32 partitions.
        state = {}

        def part1():
            den_sb = small.tile([128, 512], F32, tag="den_sb", name="den_sb_t")
            nc.vector.tensor_copy(out=den_sb[:], in_=den_[:, :])
            den_dr = dram_pool.tile([4, 512], F32, tag="den_dr", name="den_dr")
            for h2 in range(4):
                nc.sync.dma_start(
                    out=den_dr[h2 : h2 + 1, :], in_=den_sb[32 * h2 : 32 * h2 + 1, :]
                )
            rden32 = small.tile([32, 64], F32, tag="rden32", name="rden32_t")
            nc.sync.dma_start(
                out=rden32[:], in_=den_dr[:].rearrange("f (p j) -> (f p) j", j=64)
            )
            state["rden32"] = rden32

        def part2():
            rden32 = state["rden32"]
            nc.vector.reciprocal(out=rden32[:], in_=rden32[:])
            rden_dr = dram_pool.tile([1, 2048], F32, tag="rden_dr", name="rden_dr")
            nc.sync.dma_start(
                out=rden_dr[:].rearrange("one (p j) -> (one p) j", j=64), in_=rden32[:]
            )
            rdb = small.tile([32, 2048], F32, tag="rdb", name="rdb_t")
            nc.sync.dma_start(out=rdb[:], in_=rden_dr[0:1, :].to_broadcast([32, 2048]))
            state["rdb"] = rdb

        def part3():
            rdb = state["rdb"]
            for h2 in range(4):
                nc.vector.tensor_mul(
                    out=otn_sb[b_][quad_][32 * h2 : 32 * (h2 + 1), 512 * qi_ : 512 * (qi_ + 1)],
                    in0=ot_[32 * h2 : 32 * (h2 + 1), :],
                    in1=rdb[:, 512 * h2 : 512 * (h2 + 1)],
                )

        return [part1, part2, part3]

    eb3 = eb_sb[:].rearrange("p (h w) -> p h w", w=GW)

    def stage_c_group(b, ct, q2, po):
        for ch in range(2):
            nc.tensor.matmul(
                po[:, 0:512],
                lhsT=wo_sb[ch][:, 128 * ct : 128 * (ct + 1)],
                rhs=otn_sb[b][ch][:, 512 * q2 : 512 * (q2 + 1)],
                start=(ch == 0),
                stop=(ch == 1),
            )
        ob = outp.tile([128, 512], F32, tag="ob", name="ob_t")
        nc.scalar.activation(
            out=ob[:], in_=po[:, 0:512], func=AF.Identity, bias=bo_sb[ct][:], scale=1.0
        )
        nc.sync.dma_start(
            out=out[b, 128 * ct : 128 * (ct + 1), 512 * q2 : 512 * (q2 + 1)],
            in_=ob[:],
        )

    # batch 0's output projection is dripped into the final attention block
    c_parts = [
        (lambda ct=ct, q2=q2: stage_c_group(
            0, ct, q2, ps_den.tile([128, 512], F32, tag="den", name="den_ps")))
        for ct in range(2) for q2 in range(2)
    ]

    for quad in range(2):
        for b in range(B_LOC):
            if b == 1:
                while deferred_proj:
                    deferred_proj.pop(0)()
            for qi in range(2):  # query chunk of 512
                # O^T for the 4 heads (rows 32*h2, col-tiled) and their
                # denominators (broadcast over each 32-row group), accumulated
                # over kt.  One bank each, double-buffered across qi.
                ot_ps = ps_ot.tile([128, 512], F32, tag="otden", name="ot_ps")
                den_ps = ps_den.tile([128, 512], F32, tag="den", name="den_ps")
                for kt in range(8):
                    st = ps_st.tile([128, 2048], F32, tag="st", name="st_ps")
                    for h2 in range(4):
                        nc.tensor.matmul(
                            st[:, 512 * h2 : 512 * (h2 + 1)],
                            lhsT=kT_sb[b][quad][32 * h2 : 32 * (h2 + 1), 128 * kt : 128 * (kt + 1)],
                            rhs=qT_sb[b][quad][32 * h2 : 32 * (h2 + 1), 512 * qi : 512 * (qi + 1)],
                            start=True,
                            stop=True,
                            tile_position=(32 * h2, 0),
                        )
                    # exp(S+B) = exp(S) * exp(B): one wide exp on ScalarE
                    # (PSUM->SBUF, bf16), one strided bf16 2x multiply on
                    # VectorE against the 4 heads' exp-of-bias slices.
                    se = stexp_pool.tile([128, 2048], BF16, tag="se", name="se_t")
                    nc.scalar.activation(out=se[:], in_=st[:], func=AF.Exp)
                    off = 896 - 128 * kt + 512 * qi
                    nc.vector.tensor_mul(
                        out=se[:].rearrange("p (h q) -> p h q", h=4),
                        in0=se[:].rearrange("p (h q) -> p h q", h=4),
                        in1=eb3[:, 4 * quad : 4 * quad + 4, off : off + 512],
                    )
                    if pending_pv is not None:
                        _emit_pv(pending_pv)
                    if kt in (0, 2, 5) and norm_parts:
                        norm_parts.pop(0)()
                    if quad == 0 and b == 0 and deferred_proj:
                        deferred_proj.pop(0)()
                    if quad == 1 and b == 1 and qi == 1 and kt in (0, 2, 4, 6) and c_parts:
                        c_parts.pop(0)()
                    pending_pv = (ot_ps, den_ps, b, kt, se, quad, qi, kt == 0, kt == 7)
                while norm_parts:
                    norm_parts.pop(0)()
                norm_parts = _make_norm(ot_ps, den_ps, quad, b, qi)
    _emit_pv(pending_pv)
    while norm_parts:
        norm_parts.pop(0)()

    # ---------- stage C tail: leftover b0 groups, then batch 1 ----------
    while c_parts:
        c_parts.pop(0)()
    pi = 0
    for ct in range(2):
        for q2 in range(2):
            stage_c_group(1, ct, q2, proj_psum(pi))
            pi += 1


def build():
    nc = bacc.Bacc("TRN2", target_bir_lowering=False, debug=False, num_devices=N_CORES)
    io = {
        "x": nc.dram_tensor("x", [B_LOC, C, N], BF16, kind="ExternalInput").ap(),
        "wqT": nc.dram_tensor("wqT", [C, C], BF16, kind="ExternalInput").ap(),
        "wkT": nc.dram_tensor("wkT", [C, C], BF16, kind="ExternalInput").ap(),
        "wvT": nc.dram_tensor("wvT", [C, C], BF16, kind="ExternalInput").ap(),
        "woT": nc.dram_tensor("woT", [C, C], BF16, kind="ExternalInput").ap(),
        "bo": nc.dram_tensor("bo", [C, 1], F32, kind="ExternalInput").ap(),
        "eb": nc.dram_tensor("eb", [HEADS, 128, GW], BF16, kind="ExternalInput").ap(),
        "out": nc.dram_tensor("out", [B_LOC, C, N], F32, kind="ExternalOutput").ap(),
    }
    with tile.TileContext(nc) as tc:
        _emit(tc, io)
    nc.compile()
    return nc


_CACHE: dict = {}


def _get_nc():
    if "nc" not in _CACHE:
        _CACHE["nc"] = build()
    return _CACHE["nc"]


def make_in_maps(x, Wq, Wk, Wv, Wo, bo, rel_bias, rel_idx=None):
    """Host-side sharding/layout prep. Returns per-core input maps."""
    import ml_dtypes

    bf16 = ml_dtypes.bfloat16
    x = np.asarray(x, np.float32)
    b, c, H, W = x.shape
    assert (b, c, H * W) == (B_LOC * N_CORES, C, N)
    xr = np.ascontiguousarray(x.reshape(b, c, N).astype(bf16))
    wqT = np.ascontiguousarray(np.asarray(Wq, np.float32).T.astype(bf16))
    wkT = np.ascontiguousarray((np.asarray(Wk, np.float32) * SCALE).T.astype(bf16))
    wvT = np.ascontiguousarray(np.asarray(Wv, np.float32).T.astype(bf16))
    woT = np.ascontiguousarray(np.asarray(Wo, np.float32).T.astype(bf16))
    bo2 = np.ascontiguousarray(np.asarray(bo, np.float32).reshape(C, 1))
    rb = np.asarray(rel_bias, np.float32)
    idx = G0 + np.arange(128)[:, None] - np.arange(GW)[None, :]
    ebmat = np.ascontiguousarray(np.exp(rb[:, idx]).astype(bf16))  # [8, 128, GW]
    shared = dict(wqT=wqT, wkT=wkT, wvT=wvT, woT=woT, bo=bo2, eb=ebmat)
    return [
        dict(x=np.ascontiguousarray(xr[B_LOC * i : B_LOC * (i + 1)]), **shared)
        for i in range(N_CORES)
    ]


def _install_ntff_hook_shim():
    """bass_utils fetches the axon NTFF hook via antenv.axon_hooks, which this
    image's antenv lacks; synthesize it from trn_agent_boot's ctypes hook."""
    import sys
    import types

    try:
        from antenv.axon_hooks import get_axon_ntff_profile_hook  # noqa: F401

        return
    except ImportError:
        pass
    hook = None
    try:
        from trn_agent_boot.trn_boot import _ntff_profile_via_ctypes

        hook = _ntff_profile_via_ctypes("/opt/axon/libaxon_pjrt.so")
    except Exception:
        pass
    mod = types.ModuleType("antenv.axon_hooks")
    state = {"hook": hook}
    mod.get_axon_ntff_profile_hook = lambda: state["hook"]
    mod.set_axon_ntff_profile_hook = lambda h: state.__setitem__("hook", h)
    sys.modules["antenv.axon_hooks"] = mod


def run(inputs: dict, trace: bool = False):
    """Run on the 8 cores; returns (full_output, BassKernelResults)."""
    if trace:
        _install_ntff_hook_shim()
    in_maps = make_in_maps(**inputs)
    nc = _get_nc()
    res = bass_utils.run_bass_kernel_spmd(
        nc, in_maps, core_ids=list(range(N_CORES)), trace=trace
    )
    outs = np.stack([res.results[i]["out"] for i in range(N_CORES)])
    out = outs.reshape(B_LOC * N_CORES, C, 32, 32)
    return out, res


def kernel(**inputs) -> np.ndarray:
    out, _ = run(inputs)
    return out



# revision 3
# speedup vs baseline: 1.0694x; 1.0694x over previous
"""CoAtNet relative attention kernel for Trainium2 (Bass/Tile), 8 NeuronCores.

Problem (per full input):
  x [16, 256, 32, 32] f32; Wq/Wk/Wv [256, 256]; Wo [256, 256]; bo [256];
  rel_bias [8, 3969]; rel_idx [1024, 1024] int32 (static pattern).
  out[b] = softmax(q k^T / sqrt(d) + bias) v  projected back, heads=8, d=32.

Sharding: data-parallel over batch — each of the 8 cores handles 2 batches
with identical programs (SPMD, no collectives).

Key structural facts used:
  * rel_idx[p, q] == (q - p) + 1056 exactly (the reference's quirky *W stride
    collapses the 2D relative index to 1D Toeplitz).  So the [1024, 1024]
    bias matrix per head is bias[p, q] = rel_bias[h, q - p + 1056] and any
    [128, width] tile of it (keys on partitions) is a contiguous slice of a
    small "sheared" tile  G[h, i, j'] = rel_bias[h, 1952 + i - j']  of shape
    [128, 1920].  No gather on device at all.  The bias is applied as
    exp(S+B) = exp(S) * exp(B) with exp(B) precomputed, so the application
    is a bf16 2x-mode multiply instead of an fp32 1x add.
  * Everything is computed in "transposed" layout so no transposes are ever
    needed: x arrives as [c, n] per batch; Q^T/K^T = W @ x are [d_all, n];
    scores are built as S^T [keys, queries]; P@V uses lhsT = V directly;
    and the final projection produces out^T [c, n], exactly the output
    memory layout.
  * The kernel is ACT(exp)-throughput-bound: 16.8M exps/core at 1 elem/
    lane/cycle @1.2GHz is a ~110us floor.  The schedule is built so the
    ScalarE never waits: each strip's exp is SPLIT into two ACTIVATEs over
    separate PSUM tiles (st_lo banks 0-1, st_hi banks 2-3) so the next
    strip's score matmuls can overwrite the low banks while ACT still
    processes the high banks (the single-buffered [128,2048] fp32 score
    tile cannot be double-buffered: TRN2 matmuls write fp32-only and PSUM
    has just 8 banks).  PV/den matmuls run TWO strips behind the score
    matmuls so a DVE-gated PV never head-of-line blocks ready ST work in
    the PE FIFO.  All projection / output-projection / softmax-
    normalization work is dripped into per-strip slack slots on the other
    engines.
"""

import numpy as np
from collections import deque
from contextlib import ExitStack

import concourse.bass as bass
import concourse.bacc as bacc
import concourse.mybir as mybir
import concourse.tile as tile
from concourse import bass_utils
from concourse._compat import with_exitstack

HEADS = 8
D = 32  # head dim
C = 256  # channels = heads * D
N = 1024  # tokens = 32 * 32
B_LOC = 2  # batches per core
N_CORES = 8
SCALE = D ** -0.5
GW = 1920  # sheared bias tile width
G0 = 1952  # G[h, i, j'] = rel_bias[h, G0 + i - j']

F32 = mybir.dt.float32
BF16 = mybir.dt.bfloat16
AF = mybir.ActivationFunctionType


@with_exitstack
def _emit(ctx: ExitStack, tc: tile.TileContext, io: dict):
    nc = tc.nc
    x, wqT, wkT, wvT, woT, bo, eb, out = (
        io[k] for k in ("x", "wqT", "wkT", "wvT", "woT", "bo", "eb", "out")
    )

    persist = ctx.enter_context(tc.tile_pool(name="persist", bufs=1))
    stexp_pool = ctx.enter_context(tc.tile_pool(name="stexp", bufs=4))
    small = ctx.enter_context(tc.tile_pool(name="small", bufs=2))
    outp = ctx.enter_context(tc.tile_pool(name="outp", bufs=4))
    dram_pool = ctx.enter_context(tc.tile_pool(name="dram", bufs=2, space="DRAM"))
    # PSUM budget (8 banks): st_lo 2 + st_hi 2 + ot 2x1 + den 1x1 + drip 1.
    ps_lo = ctx.enter_context(tc.tile_pool(name="ps_lo", bufs=1, space="PSUM"))
    ps_hi = ctx.enter_context(tc.tile_pool(name="ps_hi", bufs=1, space="PSUM"))
    ps_ot = ctx.enter_context(tc.tile_pool(name="ps_ot", bufs=2, space="PSUM"))
    ps_den = ctx.enter_context(tc.tile_pool(name="ps_den", bufs=1, space="PSUM"))
    ps_drip = ctx.enter_context(tc.tile_pool(name="ps_drip", bufs=1, space="PSUM"))

    # ---------- DMAs: everything in flight up front ----------
    # eb quad0 slices first (needed by the very first bias multiply).
    eb_sb = persist.tile([128, HEADS * GW], BF16, tag="eb", name="eb_sb")
    for h in range(4):
        nc.sync.dma_start(out=eb_sb[:, GW * h : GW * (h + 1)], in_=eb[h])
    x_sb = [[persist.tile([128, N], BF16, tag=f"x{b}_{cc}", name=f"x{b}_{cc}") for cc in range(2)] for b in range(B_LOC)]
    for cc in range(2):
        nc.sync.dma_start(out=x_sb[0][cc][:], in_=x[0, 128 * cc : 128 * (cc + 1), :])
    wq_sb, wk_sb, wv_sb, wo_sb = [], [], [], []
    for cc in range(2):
        for lst, src, nm in (
            (wq_sb, wqT, "wq"),
            (wk_sb, wkT, "wk"),
            (wv_sb, wvT, "wv"),
            (wo_sb, woT, "wo"),
        ):
            t = persist.tile([128, C], BF16, tag=f"{nm}{cc}", name=f"{nm}{cc}")
            nc.sync.dma_start(out=t[:], in_=src[128 * cc : 128 * (cc + 1), :])
            lst.append(t)
    bo_sb = []
    for cc in range(2):
        t = persist.tile([128, 1], F32, tag=f"bo{cc}", name=f"bo{cc}")
        nc.sync.dma_start(out=t[:], in_=bo[128 * cc : 128 * (cc + 1), :])
        bo_sb.append(t)
    for h in range(4, HEADS):
        nc.sync.dma_start(out=eb_sb[:, GW * h : GW * (h + 1)], in_=eb[h])
    for cc in range(2):
        nc.sync.dma_start(out=x_sb[1][cc][:], in_=x[1, 128 * cc : 128 * (cc + 1), :])
    ones32_sb = persist.tile([128, 32], BF16, tag="ones32", name="ones32")
    nc.vector.memset(ones32_sb[:], 1.0)
    # warm up the exp table set (~2.7us ACT_TABLE_LOAD) under the prologue
    warm = small.tile([1, 32], F32, tag="warm", name="warm_t")
    nc.scalar.activation(out=warm[:], in_=ones32_sb[0:1, :], func=AF.Exp)

    # ---------- persistent stage-A outputs ----------
    qT_sb = [[persist.tile([128, N], BF16, tag=f"qT{b}_{oc}", name=f"qT{b}_{oc}") for oc in range(2)] for b in range(B_LOC)]
    kT_sb = [[persist.tile([128, N], BF16, tag=f"kT{b}_{oc}", name=f"kT{b}_{oc}") for oc in range(2)] for b in range(B_LOC)]
    # v: [n, o] layout, 8 row tiles of 128 tokens, ones column per head
    # (33 cols/head) so P@V emits the softmax denominator via ones32 matmuls.
    v_sb = [[persist.tile([128, 33 * HEADS], BF16, tag=f"v{b}_{nt}", name=f"v{b}_{nt}") for nt in range(8)] for b in range(B_LOC)]
    otn_sb = [[persist.tile([128, N], BF16, tag=f"otn{b}_{ch}", name=f"otn{b}_{ch}") for ch in range(2)] for b in range(B_LOC)]

    def emit_qk_group(b, oc, nc2, w_sb, dst, pool_tile):
        for cc in range(2):
            nc.tensor.matmul(
                pool_tile[:, 0:512],
                lhsT=w_sb[cc][:, 128 * oc : 128 * (oc + 1)],
                rhs=x_sb[b][cc][:, 512 * nc2 : 512 * (nc2 + 1)],
                start=(cc == 0),
                stop=(cc == 1),
            )
        nc.vector.tensor_copy(
            out=dst[b][oc][:, 512 * nc2 : 512 * (nc2 + 1)], in_=pool_tile[:, 0:512]
        )

    def emit_v_group(b, nt, pool_tile):
        for cc in range(2):
            nc.tensor.matmul(
                pool_tile[:, 0:C],
                lhsT=x_sb[b][cc][:, 128 * nt : 128 * (nt + 1)],
                rhs=wv_sb[cc][:],
                start=(cc == 0),
                stop=(cc == 1),
            )
        v33 = v_sb[b][nt][:].rearrange("p (h w) -> p h w", w=33)
        nc.vector.tensor_copy(
            out=v33[:, :, 0:32], in_=pool_tile[:, 0:C].rearrange("p (h w) -> p h w", w=32)
        )
        nc.vector.memset(v33[:, :, 32:33], 1.0)

    def stage_c_group(b, ct, q2, pool_tile):
        for ch in range(2):
            nc.tensor.matmul(
                pool_tile[:, 0:512],
                lhsT=wo_sb[ch][:, 128 * ct : 128 * (ct + 1)],
                rhs=otn_sb[b][ch][:, 512 * q2 : 512 * (q2 + 1)],
                start=(ch == 0),
                stop=(ch == 1),
            )
        ob = outp.tile([128, 512], F32, tag="ob", name="ob_t")
        nc.vector.tensor_scalar_add(out=ob[:], in0=pool_tile[:, 0:512], scalar1=bo_sb[ct][:])
        nc.sync.dma_start(
            out=out[b, 128 * ct : 128 * (ct + 1), 512 * q2 : 512 * (q2 + 1)],
            in_=ob[:],
        )

    # ---------- prologue stage A: just enough for strip 0 ----------
    ot_prol_q = ps_ot.tile([128, 512], F32, tag="ot", name="ot_ps")
    emit_qk_group(0, 0, 0, wq_sb, qT_sb, ot_prol_q)
    ot_prol_k = ps_ot.tile([128, 512], F32, tag="ot", name="ot_ps")
    emit_qk_group(0, 0, 0, wk_sb, kT_sb, ot_prol_k)
    den_prol_v = ps_den.tile([128, 512], F32, tag="den", name="den_ps")
    emit_v_group(0, 0, den_prol_v)

    # ---------- drip schedules ----------
    def drip_tile():
        return ps_drip.tile([128, 512], F32, tag="drip", name="drip_ps")

    proj_drips = deque()
    # qk(b0, oc0, nc2=1): needed by strip 8 (qi=1 of block 0)
    for w_sb, dst in ((wq_sb, qT_sb), (wk_sb, kT_sb)):
        proj_drips.append(lambda w_sb=w_sb, dst=dst: emit_qk_group(0, 0, 1, w_sb, dst, drip_tile()))
    # v(b0, 1..7): v[nt] needed by PVden at strip nt+2
    for nt in range(1, 8):
        proj_drips.append(lambda nt=nt: emit_v_group(0, nt, drip_tile()))
    # qk(b1, oc0): needed by strips 16/24
    for nc2 in range(2):
        for w_sb, dst in ((wq_sb, qT_sb), (wk_sb, kT_sb)):
            proj_drips.append(lambda nc2=nc2, w_sb=w_sb, dst=dst: emit_qk_group(1, 0, nc2, w_sb, dst, drip_tile()))
    # v(b1, 0..7): needed by strips 18..25
    for nt in range(8):
        proj_drips.append(lambda nt=nt: emit_v_group(1, nt, drip_tile()))
    # qk(b0, oc1): needed by strip 32
    for nc2 in range(2):
        for w_sb, dst in ((wq_sb, qT_sb), (wk_sb, kT_sb)):
            proj_drips.append(lambda nc2=nc2, w_sb=w_sb, dst=dst: emit_qk_group(0, 1, nc2, w_sb, dst, drip_tile()))
    # qk(b1, oc1): needed by strip 48
    for nc2 in range(2):
        for w_sb, dst in ((wq_sb, qT_sb), (wk_sb, kT_sb)):
            proj_drips.append(lambda nc2=nc2, w_sb=w_sb, dst=dst: emit_qk_group(1, 1, nc2, w_sb, dst, drip_tile()))

    # stage-C drips: strip -> thunk.  C(b, ct, q2) needs otn[b][*][:, 512q2:]
    # from both quads; quad1 norms land ~4 strips into the following qi block.
    c_drips = {}
    for i, (ct,) in enumerate([(0,), (1,)]):
        c_drips[45 + i] = lambda ct=ct: stage_c_group(0, ct, 0, drip_tile())
        c_drips[53 + i] = lambda ct=ct: stage_c_group(0, ct, 1, drip_tile())
        c_drips[60 + i] = lambda ct=ct: stage_c_group(1, ct, 0, drip_tile())

    # ---------- softmax normalization (per qi block) ----------
    eb3 = eb_sb[:].rearrange("p (h w) -> p h w", w=GW)

    def _make_norm(ot_, den_, b_, quad_, qi_):
        state = {}

        def part1():
            # den rows 32h2 hold head h2's denominators (32 identical rows).
            den_sb = small.tile([128, 512], F32, tag="den_sb", name="den_sb_t")
            nc.vector.tensor_copy(out=den_sb[:], in_=den_[:])
            den_dr = dram_pool.tile([4, 512], F32, tag="den_dr", name="den_dr")
            for h2 in range(4):
                nc.sync.dma_start(
                    out=den_dr[h2 : h2 + 1, :], in_=den_sb[32 * h2 : 32 * h2 + 1, :]
                )
            state["den_dr"] = den_dr

        def part2():
            # reciprocal on [32, 64] to use 32 lanes (8 cyc/elem iterative op)
            den_dr = state["den_dr"]
            rden32 = small.tile([32, 64], F32, tag="rden32", name="rden32_t")
            nc.sync.dma_start(
                out=rden32[:], in_=den_dr[:].rearrange("f (p j) -> (f p) j", j=64)
            )
            nc.vector.reciprocal(out=rden32[:], in_=rden32[:])
            rden_dr = dram_pool.tile([4, 512], F32, tag="rden_dr", name="rden_dr")
            nc.sync.dma_start(
                out=rden_dr[:].rearrange("f (p j) -> (f p) j", j=64), in_=rden32[:]
            )
            state["rden_dr"] = rden_dr

        def part3():
            # broadcast per-head reciprocal rows to the [128, 512] layout
            # matching ot rows, then ONE normalize mul for all 4 heads.
            rden_dr = state["rden_dr"]
            rdb = small.tile([128, 512], F32, tag="rdb", name="rdb_t")
            for h2 in range(4):
                nc.sync.dma_start(
                    out=rdb[32 * h2 : 32 * (h2 + 1), :],
                    in_=rden_dr[h2 : h2 + 1, :].to_broadcast([32, 512]),
                )
            nc.vector.tensor_mul(
                out=otn_sb[b_][quad_][:, 512 * qi_ : 512 * (qi_ + 1)],
                in0=ot_[:],
                in1=rdb[:],
            )

        return [part1, part2, part3]

    # ---------- stage B: 64 strips, lag-2 software pipeline ----------
    def emit_pvden(args):
        ot_, den_, b_, quad_, qi_, kt_, se_, first, last = args
        for h2 in range(4):
            nc.tensor.matmul(
                ot_[32 * h2 : 32 * (h2 + 1), :],
                lhsT=v_sb[b_][kt_][:, 33 * (4 * quad_ + h2) : 33 * (4 * quad_ + h2) + 32],
                rhs=se_[:, 512 * h2 : 512 * (h2 + 1)],
                start=first,
                stop=last,
                tile_position=(0, 32 * h2),
                skip_group_check=True,
            )
        for h2 in range(4):
            nc.tensor.matmul(
                den_[32 * h2 : 32 * (h2 + 1), :],
                lhsT=ones32_sb[:],
                rhs=se_[:, 512 * h2 : 512 * (h2 + 1)],
                start=first,
                stop=last,
                tile_position=(0, 32 * h2),
                skip_group_check=True,
            )
        if last:
            norm_parts.extend(_make_norm(ot_, den_, b_, quad_, qi_))

    BLOCKS = [(0, 0), (0, 1), (1, 0), (1, 1)]  # (quad, b)
    pending = deque()
    norm_parts = deque()
    block_acc = {}  # (qi,) accumulators for the current block

    for s in range(64):
        quad, b = BLOCKS[s // 16]
        qi = (s // 8) % 2
        kt = s % 8
        # norm part for a completed qi block (emitted BEFORE this strip's
        # lag-2 PVden so the den bank WAR resolves without a stall)
        if norm_parts:
            norm_parts.popleft()()
        if kt == 0:
            block_acc = (
                ps_ot.tile([128, 512], F32, tag="ot", name="ot_ps"),
                ps_den.tile([128, 512], F32, tag="den", name="den_ps"),
            )
        ot_cur, den_cur = block_acc
        st_lo = ps_lo.tile([128, 1024], F32, tag="stlo", name="stlo_ps")
        st_hi = ps_hi.tile([128, 1024], F32, tag="sthi", name="sthi_ps")
        se = stexp_pool.tile([128, 2048], BF16, tag="se", name="se_t")
        for h2 in range(4):
            dst = st_lo if h2 < 2 else st_hi
            nc.tensor.matmul(
                dst[:, 512 * (h2 % 2) : 512 * (h2 % 2 + 1)],
                lhsT=kT_sb[b][quad][32 * h2 : 32 * (h2 + 1), 128 * kt : 128 * (kt + 1)],
                rhs=qT_sb[b][quad][32 * h2 : 32 * (h2 + 1), 512 * qi : 512 * (qi + 1)],
                start=True,
                stop=True,
                tile_position=(32 * h2, 0),
            )
        nc.scalar.activation(out=se[:, 0:1024], in_=st_lo[:], func=AF.Exp)
        nc.scalar.activation(out=se[:, 1024:2048], in_=st_hi[:], func=AF.Exp)
        off = 896 - 128 * kt + 512 * qi
        nc.vector.tensor_mul(
            out=se[:].rearrange("p (h q) -> p h q", h=4),
            in0=se[:].rearrange("p (h q) -> p h q", h=4),
            in1=eb3[:, 4 * quad : 4 * quad + 4, off : off + 512],
        )
        pending.append((ot_cur, den_cur, b, quad, qi, kt, se, kt == 0, kt == 7))
        if len(pending) > 2:
            emit_pvden(pending.popleft())
        if s in c_drips:
            c_drips[s]()
        elif proj_drips:
            proj_drips.popleft()()

    # ---------- tail ----------
    while pending:
        emit_pvden(pending.popleft())
    while norm_parts:
        norm_parts.popleft()()
    for ct in range(2):
        stage_c_group(1, ct, 1, drip_tile())


def build():
    nc = bacc.Bacc("TRN2", target_bir_lowering=False, debug=False, num_devices=N_CORES)
    io = {
        "x": nc.dram_tensor("x", [B_LOC, C, N], BF16, kind="ExternalInput").ap(),
        "wqT": nc.dram_tensor("wqT", [C, C], BF16, kind="ExternalInput").ap(),
        "wkT": nc.dram_tensor("wkT", [C, C], BF16, kind="ExternalInput").ap(),
        "wvT": nc.dram_tensor("wvT", [C, C], BF16, kind="ExternalInput").ap(),
        "woT": nc.dram_tensor("woT", [C, C], BF16, kind="ExternalInput").ap(),
        "bo": nc.dram_tensor("bo", [C, 1], F32, kind="ExternalInput").ap(),
        "eb": nc.dram_tensor("eb", [HEADS, 128, GW], BF16, kind="ExternalInput").ap(),
        "out": nc.dram_tensor("out", [B_LOC, C, N], F32, kind="ExternalOutput").ap(),
    }
    with tile.TileContext(nc) as tc:
        _emit(tc, io)
    nc.compile()
    return nc


_CACHE: dict = {}


def _get_nc():
    if "nc" not in _CACHE:
        _CACHE["nc"] = build()
    return _CACHE["nc"]


def make_in_maps(x, Wq, Wk, Wv, Wo, bo, rel_bias, rel_idx=None):
    """Host-side sharding/layout prep. Returns per-core input maps."""
    import ml_dtypes

    bf16 = ml_dtypes.bfloat16
    x = np.asarray(x, np.float32)
    b, c, H, W = x.shape
    assert (b, c, H * W) == (B_LOC * N_CORES, C, N)
    xr = np.ascontiguousarray(x.reshape(b, c, N).astype(bf16))
    wqT = np.ascontiguousarray(np.asarray(Wq, np.float32).T.astype(bf16))
    wkT = np.ascontiguousarray((np.asarray(Wk, np.float32) * SCALE).T.astype(bf16))
    wvT = np.ascontiguousarray(np.asarray(Wv, np.float32).T.astype(bf16))
    woT = np.ascontiguousarray(np.asarray(Wo, np.float32).T.astype(bf16))
    bo2 = np.ascontiguousarray(np.asarray(bo, np.float32).reshape(C, 1))
    rb = np.asarray(rel_bias, np.float32)
    idx = G0 + np.arange(128)[:, None] - np.arange(GW)[None, :]
    ebmat = np.ascontiguousarray(np.exp(rb[:, idx]).astype(bf16))  # [8, 128, GW]
    shared = dict(wqT=wqT, wkT=wkT, wvT=wvT, woT=woT, bo=bo2, eb=ebmat)
    return [
        dict(x=np.ascontiguousarray(xr[B_LOC * i : B_LOC * (i + 1)]), **shared)
        for i in range(N_CORES)
    ]


def _install_ntff_hook_shim():
    """bass_utils fetches the axon NTFF hook via antenv.axon_hooks, which this
    image's antenv lacks; synthesize it from trn_agent_boot's ctypes hook."""
    import sys
    import types

    try:
        from antenv.axon_hooks import get_axon_ntff_profile_hook  # noqa: F401

        return
    except ImportError:
        pass
    hook = None
    try:
        from trn_agent_boot.trn_boot import _ntff_profile_via_ctypes

        hook = _ntff_profile_via_ctypes("/opt/axon/libaxon_pjrt.so")
    except Exception:
        pass
    mod = types.ModuleType("antenv.axon_hooks")
    state = {"hook": hook}
    mod.get_axon_ntff_profile_hook = lambda: state["hook"]
    mod.set_axon_ntff_profile_hook = lambda h: state.__setitem__("hook", h)
    sys.modules["antenv.axon_hooks"] = mod


def run(inputs: dict, trace: bool = False):
    """Run on the 8 cores; returns (full_output, BassKernelResults)."""
    if trace:
        _install_ntff_hook_shim()
    in_maps = make_in_maps(**inputs)
    nc = _get_nc()
    res = bass_utils.run_bass_kernel_spmd(
        nc, in_maps, core_ids=list(range(N_CORES)), trace=trace
    )
    outs = np.stack([res.results[i]["out"] for i in range(N_CORES)])
    out = outs.reshape(B_LOC * N_CORES, C, 32, 32)
    return out, res


def kernel(**inputs) -> np.ndarray:
    out, _ = run(inputs)
    return out


# revision 4
# speedup vs baseline: 1.1197x; 1.0470x over previous
"""CoAtNet relative attention kernel for Trainium2 (Bass/Tile), 8 NeuronCores.

Problem (per full input):
  x [16, 256, 32, 32] f32; Wq/Wk/Wv [256, 256]; Wo [256, 256]; bo [256];
  rel_bias [8, 3969]; rel_idx [1024, 1024] int32 (static pattern).
  out[b] = softmax(q k^T / sqrt(d) + bias) v  projected back, heads=8, d=32.

Sharding: data-parallel over batch — each of the 8 cores handles 2 batches
with identical programs (SPMD, no collectives).

Key structural facts used:
  * rel_idx[p, q] == (q - p) + 1056 exactly (the reference's quirky *W stride
    collapses the 2D relative index to 1D Toeplitz).  So the [1024, 1024]
    bias matrix per head is bias[p, q] = rel_bias[h, q - p + 1056] and any
    [128, width] tile of it (keys on partitions) is a contiguous slice of a
    small "sheared" tile  G[h, i, j'] = rel_bias[h, 1952 + i - j']  of shape
    [128, 1920].  No gather on device at all.  The bias is applied as
    exp(S+B) = exp(S) * exp(B) with exp(B) precomputed, so the application
    is a bf16 2x-mode multiply instead of an fp32 1x add.
  * Everything is computed in "transposed" layout so no transposes are ever
    needed: x arrives as [c, n] per batch; Q^T/K^T = W @ x are [d_all, n];
    scores are built as S^T [keys, queries]; P@V uses lhsT = V directly;
    and the final projection produces out^T [c, n], exactly the output
    memory layout.
  * The kernel is ACT(exp)-throughput-bound: 16.8M exps/core at 1 elem/
    lane/cycle @1.2GHz is a ~110us floor.  The schedule is built so the
    ScalarE never waits: each strip's exp is SPLIT into two ACTIVATEs over
    separate PSUM tiles (st_lo banks 0-1, st_hi banks 2-3) so the next
    strip's score matmuls can overwrite the low banks while ACT still
    processes the high banks (the single-buffered [128,2048] fp32 score
    tile cannot be double-buffered: TRN2 matmuls write fp32-only and PSUM
    has just 8 banks).  PV/den matmuls run TWO strips behind the score
    matmuls so a DVE-gated PV never head-of-line blocks ready ST work in
    the PE FIFO.  All projection / output-projection / softmax-
    normalization work is dripped into per-strip slack slots on the other
    engines.
"""

import numpy as np
from collections import deque
from contextlib import ExitStack

import concourse.bass as bass
import concourse.bacc as bacc
import concourse.mybir as mybir
import concourse.tile as tile
from concourse import bass_utils
from concourse._compat import with_exitstack

HEADS = 8
D = 32  # head dim
C = 256  # channels = heads * D
N = 1024  # tokens = 32 * 32
B_LOC = 2  # batches per core
N_CORES = 8
SCALE = D ** -0.5
GW = 1920  # sheared bias tile width
G0 = 1952  # G[h, i, j'] = rel_bias[h, G0 + i - j']

F32 = mybir.dt.float32
BF16 = mybir.dt.bfloat16
AF = mybir.ActivationFunctionType


@with_exitstack
def _emit(ctx: ExitStack, tc: tile.TileContext, io: dict):
    nc = tc.nc
    x, wqT, wkT, wvT, woT, bo, eb, out = (
        io[k] for k in ("x", "wqT", "wkT", "wvT", "woT", "bo", "eb", "out")
    )

    persist = ctx.enter_context(tc.tile_pool(name="persist", bufs=1))
    stexp_pool = ctx.enter_context(tc.tile_pool(name="stexp", bufs=4))
    small = ctx.enter_context(tc.tile_pool(name="small", bufs=2))
    outp = ctx.enter_context(tc.tile_pool(name="outp", bufs=4))
    dram_pool = ctx.enter_context(tc.tile_pool(name="dram", bufs=2, space="DRAM"))
    # PSUM budget (8 banks): st_lo 2 + st_hi 2 + ot 2x1 + den 1x1 + drip 1.
    ps_lo = ctx.enter_context(tc.tile_pool(name="ps_lo", bufs=1, space="PSUM"))
    ps_hi = ctx.enter_context(tc.tile_pool(name="ps_hi", bufs=1, space="PSUM"))
    ps_ot = ctx.enter_context(tc.tile_pool(name="ps_ot", bufs=2, space="PSUM"))
    ps_den = ctx.enter_context(tc.tile_pool(name="ps_den", bufs=1, space="PSUM"))
    ps_drip = ctx.enter_context(tc.tile_pool(name="ps_drip", bufs=1, space="PSUM"))

    # ---------- DMAs: everything in flight up front ----------
    # eb quad0 slices first (needed by the very first bias multiply).
    eb_sb = persist.tile([128, HEADS * GW], BF16, tag="eb", name="eb_sb")
    for h in range(4):
        nc.sync.dma_start(out=eb_sb[:, GW * h : GW * (h + 1)], in_=eb[h])
    x_sb = [[persist.tile([128, N], BF16, tag=f"x{b}_{cc}", name=f"x{b}_{cc}") for cc in range(2)] for b in range(B_LOC)]
    for cc in range(2):
        nc.sync.dma_start(out=x_sb[0][cc][:], in_=x[0, 128 * cc : 128 * (cc + 1), :])
    wq_sb, wk_sb, wv_sb, wo_sb = [], [], [], []
    for cc in range(2):
        for lst, src, nm in (
            (wq_sb, wqT, "wq"),
            (wk_sb, wkT, "wk"),
            (wv_sb, wvT, "wv"),
            (wo_sb, woT, "wo"),
        ):
            t = persist.tile([128, C], BF16, tag=f"{nm}{cc}", name=f"{nm}{cc}")
            nc.sync.dma_start(out=t[:], in_=src[128 * cc : 128 * (cc + 1), :])
            lst.append(t)
    bo_sb = []
    for cc in range(2):
        t = persist.tile([128, 1], F32, tag=f"bo{cc}", name=f"bo{cc}")
        nc.sync.dma_start(out=t[:], in_=bo[128 * cc : 128 * (cc + 1), :])
        bo_sb.append(t)
    for h in range(4, HEADS):
        nc.sync.dma_start(out=eb_sb[:, GW * h : GW * (h + 1)], in_=eb[h])
    for cc in range(2):
        nc.sync.dma_start(out=x_sb[1][cc][:], in_=x[1, 128 * cc : 128 * (cc + 1), :])
    ones32_sb = persist.tile([128, 32], BF16, tag="ones32", name="ones32")
    nc.vector.memset(ones32_sb[:], 1.0)
    # warm up the exp table set (~2.7us ACT_TABLE_LOAD) under the prologue
    warm = small.tile([1, 32], F32, tag="warm", name="warm_t")
    nc.scalar.activation(out=warm[:], in_=ones32_sb[0:1, :], func=AF.Exp)

    # ---------- persistent stage-A outputs ----------
    qT_sb = [[persist.tile([128, N], BF16, tag=f"qT{b}_{oc}", name=f"qT{b}_{oc}") for oc in range(2)] for b in range(B_LOC)]
    kT_sb = [[persist.tile([128, N], BF16, tag=f"kT{b}_{oc}", name=f"kT{b}_{oc}") for oc in range(2)] for b in range(B_LOC)]
    # v: [n, o] layout, 8 row tiles of 128 tokens, ones column per head
    # (33 cols/head) so P@V emits the softmax denominator via ones32 matmuls.
    v_sb = [[persist.tile([128, 33 * HEADS], BF16, tag=f"v{b}_{nt}", name=f"v{b}_{nt}") for nt in range(8)] for b in range(B_LOC)]
    otn_sb = [[persist.tile([128, N], BF16, tag=f"otn{b}_{ch}", name=f"otn{b}_{ch}") for ch in range(2)] for b in range(B_LOC)]

    def emit_qk_group(b, oc, nc2, w_sb, dst, pool_tile):
        for cc in range(2):
            nc.tensor.matmul(
                pool_tile[:, 0:512],
                lhsT=w_sb[cc][:, 128 * oc : 128 * (oc + 1)],
                rhs=x_sb[b][cc][:, 512 * nc2 : 512 * (nc2 + 1)],
                start=(cc == 0),
                stop=(cc == 1),
            )
        nc.vector.tensor_copy(
            out=dst[b][oc][:, 512 * nc2 : 512 * (nc2 + 1)], in_=pool_tile[:, 0:512]
        )

    def emit_v_group(b, nt, pool_tile):
        for cc in range(2):
            nc.tensor.matmul(
                pool_tile[:, 0:C],
                lhsT=x_sb[b][cc][:, 128 * nt : 128 * (nt + 1)],
                rhs=wv_sb[cc][:],
                start=(cc == 0),
                stop=(cc == 1),
            )
        v33 = v_sb[b][nt][:].rearrange("p (h w) -> p h w", w=33)
        nc.vector.tensor_copy(
            out=v33[:, :, 0:32], in_=pool_tile[:, 0:C].rearrange("p (h w) -> p h w", w=32)
        )
        nc.vector.memset(v33[:, :, 32:33], 1.0)

    def stage_c_group(b, ct, q2, pool_tile):
        for ch in range(2):
            nc.tensor.matmul(
                pool_tile[:, 0:512],
                lhsT=wo_sb[ch][:, 128 * ct : 128 * (ct + 1)],
                rhs=otn_sb[b][ch][:, 512 * q2 : 512 * (q2 + 1)],
                start=(ch == 0),
                stop=(ch == 1),
            )
        ob = outp.tile([128, 512], F32, tag="ob", name="ob_t")
        nc.vector.tensor_scalar_add(out=ob[:], in0=pool_tile[:, 0:512], scalar1=bo_sb[ct][:])
        nc.sync.dma_start(
            out=out[b, 128 * ct : 128 * (ct + 1), 512 * q2 : 512 * (q2 + 1)],
            in_=ob[:],
        )

    # ---------- prologue stage A ----------
    # Dense PE burst: all of b0's projections + b1's V.  This keeps the
    # steady-state strips at bare ST+PV+den PE load (fits the exp cadence
    # even at the cold 1.2 GHz clock) and the burst itself warms the HAM.
    def drip_tile():
        return ps_drip.tile([128, 512], F32, tag="drip", name="drip_ps")

    pi = 0

    def prol_tile():
        nonlocal pi
        pi += 1
        if pi % 3 == 0:
            return drip_tile()
        return ps_ot.tile([128, 512], F32, tag="ot", name="ot_ps")

    for oc in range(2):
        for nc2 in range(2):
            for w_sb, dst in ((wq_sb, qT_sb), (wk_sb, kT_sb)):
                emit_qk_group(0, oc, nc2, w_sb, dst, prol_tile())
    for nt in range(8):
        emit_v_group(0, nt, prol_tile())
    for nt in range(8):
        emit_v_group(1, nt, prol_tile())
    # den pool parity: an even number of ot-pool prologue requests is not
    # required (bufs rotate per request; first qi accumulator just waits on
    # whichever prologue group last used its buffer).

    # ---------- drip schedules ----------
    proj_drips = deque()
    # qk(b1): oc0 needed by strip 16, oc1 by strip 48
    for oc in range(2):
        for nc2 in range(2):
            for w_sb, dst in ((wq_sb, qT_sb), (wk_sb, kT_sb)):
                proj_drips.append(lambda oc=oc, nc2=nc2, w_sb=w_sb, dst=dst: emit_qk_group(1, oc, nc2, w_sb, dst, drip_tile()))

    # drip one group every other strip (strips 0..14 cover all 8)
    drip_strips = {2 * i: i for i in range(8)}

    # stage-C drips: strip -> thunk.  C(b, ct, q2) needs otn[b][*][:, 512q2:]
    # from both quads; quad1 norms land ~4 strips into the following qi block.
    c_drips = {}
    for i, (ct,) in enumerate([(0,), (1,)]):
        c_drips[45 + i] = lambda ct=ct: stage_c_group(0, ct, 0, drip_tile())
        c_drips[53 + i] = lambda ct=ct: stage_c_group(0, ct, 1, drip_tile())
        c_drips[60 + i] = lambda ct=ct: stage_c_group(1, ct, 0, drip_tile())

    # ---------- softmax normalization (per qi block) ----------
    eb3 = eb_sb[:].rearrange("p (h w) -> p h w", w=GW)

    def _make_norm(ot_, den_, b_, quad_, qi_):
        state = {}

        def part1():
            # den rows 32h2 hold head h2's denominators (32 identical rows).
            den_sb = small.tile([128, 512], F32, tag="den_sb", name="den_sb_t")
            nc.vector.tensor_copy(out=den_sb[:], in_=den_[:])
            den_dr = dram_pool.tile([4, 512], F32, tag="den_dr", name="den_dr")
            for h2 in range(4):
                nc.sync.dma_start(
                    out=den_dr[h2 : h2 + 1, :], in_=den_sb[32 * h2 : 32 * h2 + 1, :]
                )
            state["den_dr"] = den_dr

        def part2():
            # reciprocal on [32, 64] to use 32 lanes (8 cyc/elem iterative op)
            den_dr = state["den_dr"]
            rden32 = small.tile([32, 64], F32, tag="rden32", name="rden32_t")
            nc.sync.dma_start(
                out=rden32[:], in_=den_dr[:].rearrange("f (p j) -> (f p) j", j=64)
            )
            nc.vector.reciprocal(out=rden32[:], in_=rden32[:])
            rden_dr = dram_pool.tile([4, 512], F32, tag="rden_dr", name="rden_dr")
            nc.sync.dma_start(
                out=rden_dr[:].rearrange("f (p j) -> (f p) j", j=64), in_=rden32[:]
            )
            state["rden_dr"] = rden_dr

        def part3():
            # broadcast per-head reciprocal rows to the [128, 512] layout
            # matching ot rows, then ONE normalize mul for all 4 heads.
            rden_dr = state["rden_dr"]
            rdb = small.tile([128, 512], F32, tag="rdb", name="rdb_t")
            for h2 in range(4):
                nc.sync.dma_start(
                    out=rdb[32 * h2 : 32 * (h2 + 1), :],
                    in_=rden_dr[h2 : h2 + 1, :].to_broadcast([32, 512]),
                )
            nc.vector.tensor_mul(
                out=otn_sb[b_][quad_][:, 512 * qi_ : 512 * (qi_ + 1)],
                in0=ot_[:],
                in1=rdb[:],
            )

        return [part1, part2, part3]

    # ---------- stage B: 64 strips, lag-2 software pipeline ----------
    def emit_pvden(args):
        ot_, den_, b_, quad_, qi_, kt_, se_, first, last = args
        for h2 in range(4):
            nc.tensor.matmul(
                ot_[32 * h2 : 32 * (h2 + 1), :],
                lhsT=v_sb[b_][kt_][:, 33 * (4 * quad_ + h2) : 33 * (4 * quad_ + h2) + 32],
                rhs=se_[:, 512 * h2 : 512 * (h2 + 1)],
                start=first,
                stop=last,
                tile_position=(0, 32 * h2),
                skip_group_check=True,
            )
        for h2 in range(4):
            nc.tensor.matmul(
                den_[32 * h2 : 32 * (h2 + 1), :],
                lhsT=ones32_sb[:],
                rhs=se_[:, 512 * h2 : 512 * (h2 + 1)],
                start=first,
                stop=last,
                tile_position=(0, 32 * h2),
                skip_group_check=True,
            )
        if last:
            norm_parts.extend(_make_norm(ot_, den_, b_, quad_, qi_))

    BLOCKS = [(0, 0), (0, 1), (1, 0), (1, 1)]  # (quad, b)
    pending = deque()
    norm_parts = deque()
    block_acc = {}  # (qi,) accumulators for the current block

    for s in range(64):
        quad, b = BLOCKS[s // 16]
        qi = (s // 8) % 2
        kt = s % 8
        # norm part for a completed qi block (emitted BEFORE this strip's
        # lag-2 PVden so the den bank WAR resolves without a stall)
        if norm_parts:
            norm_parts.popleft()()
        if kt == 0:
            block_acc = (
                ps_ot.tile([128, 512], F32, tag="ot", name="ot_ps"),
                ps_den.tile([128, 512], F32, tag="den", name="den_ps"),
            )
        ot_cur, den_cur = block_acc
        st_lo = ps_lo.tile([128, 1024], F32, tag="stlo", name="stlo_ps")
        st_hi = ps_hi.tile([128, 1024], F32, tag="sthi", name="sthi_ps")
        se = stexp_pool.tile([128, 2048], BF16, tag="se", name="se_t")
        for h2 in range(4):
            dst = st_lo if h2 < 2 else st_hi
            nc.tensor.matmul(
                dst[:, 512 * (h2 % 2) : 512 * (h2 % 2 + 1)],
                lhsT=kT_sb[b][quad][32 * h2 : 32 * (h2 + 1), 128 * kt : 128 * (kt + 1)],
                rhs=qT_sb[b][quad][32 * h2 : 32 * (h2 + 1), 512 * qi : 512 * (qi + 1)],
                start=True,
                stop=True,
                tile_position=(32 * h2, 0),
            )
        nc.scalar.activation(out=se[:, 0:1024], in_=st_lo[:], func=AF.Exp)
        nc.scalar.activation(out=se[:, 1024:2048], in_=st_hi[:], func=AF.Exp)
        off = 896 - 128 * kt + 512 * qi
        nc.vector.tensor_mul(
            out=se[:].rearrange("p (h q) -> p h q", h=4),
            in0=se[:].rearrange("p (h q) -> p h q", h=4),
            in1=eb3[:, 4 * quad : 4 * quad + 4, off : off + 512],
        )
        pending.append((ot_cur, den_cur, b, quad, qi, kt, se, kt == 0, kt == 7))
        if len(pending) > 2:
            emit_pvden(pending.popleft())
        if s in c_drips:
            c_drips[s]()
        elif proj_drips:
            proj_drips.popleft()()

    # ---------- tail ----------
    while pending:
        emit_pvden(pending.popleft())
    while norm_parts:
        norm_parts.popleft()()
    for ct in range(2):
        stage_c_group(1, ct, 1, drip_tile())


def build():
    nc = bacc.Bacc("TRN2", target_bir_lowering=False, debug=False, num_devices=N_CORES)
    io = {
        "x": nc.dram_tensor("x", [B_LOC, C, N], BF16, kind="ExternalInput").ap(),
        "wqT": nc.dram_tensor("wqT", [C, C], BF16, kind="ExternalInput").ap(),
        "wkT": nc.dram_tensor("wkT", [C, C], BF16, kind="ExternalInput").ap(),
        "wvT": nc.dram_tensor("wvT", [C, C], BF16, kind="ExternalInput").ap(),
        "woT": nc.dram_tensor("woT", [C, C], BF16, kind="ExternalInput").ap(),
        "bo": nc.dram_tensor("bo", [C, 1], F32, kind="ExternalInput").ap(),
        "eb": nc.dram_tensor("eb", [HEADS, 128, GW], BF16, kind="ExternalInput").ap(),
        "out": nc.dram_tensor("out", [B_LOC, C, N], F32, kind="ExternalOutput").ap(),
    }
    with tile.TileContext(nc) as tc:
        _emit(tc, io)
    nc.compile()
    return nc


_CACHE: dict = {}


def _get_nc():
    if "nc" not in _CACHE:
        _CACHE["nc"] = build()
    return _CACHE["nc"]


def make_in_maps(x, Wq, Wk, Wv, Wo, bo, rel_bias, rel_idx=None):
    """Host-side sharding/layout prep. Returns per-core input maps."""
    import ml_dtypes

    bf16 = ml_dtypes.bfloat16
    x = np.asarray(x, np.float32)
    b, c, H, W = x.shape
    assert (b, c, H * W) == (B_LOC * N_CORES, C, N)
    xr = np.ascontiguousarray(x.reshape(b, c, N).astype(bf16))
    wqT = np.ascontiguousarray(np.asarray(Wq, np.float32).T.astype(bf16))
    wkT = np.ascontiguousarray((np.asarray(Wk, np.float32) * SCALE).T.astype(bf16))
    wvT = np.ascontiguousarray(np.asarray(Wv, np.float32).T.astype(bf16))
    woT = np.ascontiguousarray(np.asarray(Wo, np.float32).T.astype(bf16))
    bo2 = np.ascontiguousarray(np.asarray(bo, np.float32).reshape(C, 1))
    rb = np.asarray(rel_bias, np.float32)
    idx = G0 + np.arange(128)[:, None] - np.arange(GW)[None, :]
    ebmat = np.ascontiguousarray(np.exp(rb[:, idx]).astype(bf16))  # [8, 128, GW]
    shared = dict(wqT=wqT, wkT=wkT, wvT=wvT, woT=woT, bo=bo2, eb=ebmat)
    return [
        dict(x=np.ascontiguousarray(xr[B_LOC * i : B_LOC * (i + 1)]), **shared)
        for i in range(N_CORES)
    ]


def _install_ntff_hook_shim():
    """bass_utils fetches the axon NTFF hook via antenv.axon_hooks, which this
    image's antenv lacks; synthesize it from trn_agent_boot's ctypes hook."""
    import sys
    import types

    try:
        from antenv.axon_hooks import get_axon_ntff_profile_hook  # noqa: F401

        return
    except ImportError:
        pass
    hook = None
    try:
        from trn_agent_boot.trn_boot import _ntff_profile_via_ctypes

        hook = _ntff_profile_via_ctypes("/opt/axon/libaxon_pjrt.so")
    except Exception:
        pass
    mod = types.ModuleType("antenv.axon_hooks")
    state = {"hook": hook}
    mod.get_axon_ntff_profile_hook = lambda: state["hook"]
    mod.set_axon_ntff_profile_hook = lambda h: state.__setitem__("hook", h)
    sys.modules["antenv.axon_hooks"] = mod


def run(inputs: dict, trace: bool = False):
    """Run on the 8 cores; returns (full_output, BassKernelResults)."""
    if trace:
        _install_ntff_hook_shim()
    in_maps = make_in_maps(**inputs)
    nc = _get_nc()
    res = bass_utils.run_bass_kernel_spmd(
        nc, in_maps, core_ids=list(range(N_CORES)), trace=trace
    )
    outs = np.stack([res.results[i]["out"] for i in range(N_CORES)])
    out = outs.reshape(B_LOC * N_CORES, C, 32, 32)
    return out, res


def kernel(**inputs) -> np.ndarray:
    out, _ = run(inputs)
    return out


# revision 7
# speedup vs baseline: 1.1347x; 1.0135x over previous
"""CoAtNet relative attention kernel for Trainium2 (Bass/Tile), 8 NeuronCores.

Problem (per full input):
  x [16, 256, 32, 32] f32; Wq/Wk/Wv [256, 256]; Wo [256, 256]; bo [256];
  rel_bias [8, 3969]; rel_idx [1024, 1024] int32 (static pattern).
  out[b] = softmax(q k^T / sqrt(d) + bias) v  projected back, heads=8, d=32.

Sharding: data-parallel over batch — each of the 8 cores handles 2 batches
with identical programs (SPMD, no collectives).

Key structural facts used:
  * rel_idx[p, q] == (q - p) + 1056 exactly (the reference's quirky *W stride
    collapses the 2D relative index to 1D Toeplitz).  So the [1024, 1024]
    bias matrix per head is bias[p, q] = rel_bias[h, q - p + 1056] and any
    [128, width] tile of it (keys on partitions) is a contiguous slice of a
    small "sheared" tile  G[h, i, j'] = rel_bias[h, 1952 + i - j']  of shape
    [128, 1920].  No gather on device at all.  The bias is applied as
    exp(S+B) = exp(S) * exp(B) with exp(B) precomputed, so the application
    is a bf16 2x-mode multiply instead of an fp32 1x add.
  * Everything is computed in "transposed" layout so no transposes are ever
    needed: x arrives as [c, n] per batch; Q^T/K^T = W @ x are [d_all, n];
    scores are built as S^T [keys, queries]; P@V uses lhsT = V directly;
    and the final projection produces out^T [c, n], exactly the output
    memory layout.
  * The kernel is ACT(exp)-throughput-bound: 16.8M exps/core at 1 elem/
    lane/cycle @1.2GHz is a ~110us floor.  The schedule is built so the
    ScalarE never waits: each strip's exp is SPLIT into two ACTIVATEs over
    separate PSUM tiles (st_lo banks 0-1, st_hi banks 2-3) so the next
    strip's score matmuls can overwrite the low banks while ACT still
    processes the high banks (the single-buffered [128,2048] fp32 score
    tile cannot be double-buffered: TRN2 matmuls write fp32-only and PSUM
    has just 8 banks).  PV/den matmuls run TWO strips behind the score
    matmuls so a DVE-gated PV never head-of-line blocks ready ST work in
    the PE FIFO.  All projection / output-projection / softmax-
    normalization work is dripped into per-strip slack slots on the other
    engines.
"""

import numpy as np
from collections import deque
from contextlib import ExitStack

import concourse.bass as bass
import concourse.bacc as bacc
import concourse.mybir as mybir
import concourse.tile as tile
from concourse import bass_utils
from concourse._compat import with_exitstack

HEADS = 8
D = 32  # head dim
C = 256  # channels = heads * D
N = 1024  # tokens = 32 * 32
B_LOC = 2  # batches per core
N_CORES = 8
SCALE = D ** -0.5
GW = 1920  # sheared bias tile width
G0 = 1952  # G[h, i, j'] = rel_bias[h, G0 + i - j']

F32 = mybir.dt.float32
BF16 = mybir.dt.bfloat16
AF = mybir.ActivationFunctionType


@with_exitstack
def _emit(ctx: ExitStack, tc: tile.TileContext, io: dict):
    nc = tc.nc
    x, wqT, wkT, wvT, woT, bo, eb, out = (
        io[k] for k in ("x", "wqT", "wkT", "wvT", "woT", "bo", "eb", "out")
    )

    persist = ctx.enter_context(tc.tile_pool(name="persist", bufs=1))
    stexp_pool = ctx.enter_context(tc.tile_pool(name="stexp", bufs=4))
    small = ctx.enter_context(tc.tile_pool(name="small", bufs=2))
    outp = ctx.enter_context(tc.tile_pool(name="outp", bufs=4))
    dram_pool = ctx.enter_context(tc.tile_pool(name="dram", bufs=2, space="DRAM"))
    # PSUM budget (8 banks): st_lo 2 + st_hi 2 + ot 2x1 + den 1x1 + drip 1.
    ps_lo = ctx.enter_context(tc.tile_pool(name="ps_lo", bufs=1, space="PSUM"))
    ps_hi = ctx.enter_context(tc.tile_pool(name="ps_hi", bufs=1, space="PSUM"))
    ps_ot = ctx.enter_context(tc.tile_pool(name="ps_ot", bufs=2, space="PSUM"))
    ps_den = ctx.enter_context(tc.tile_pool(name="ps_den", bufs=1, space="PSUM"))
    ps_drip = ctx.enter_context(tc.tile_pool(name="ps_drip", bufs=1, space="PSUM"))

    # ---------- DMAs: everything in flight up front ----------
    # eb quad0 slices first (needed by the very first bias multiply).
    eb_sb = persist.tile([128, HEADS * GW], BF16, tag="eb", name="eb_sb")
    for h in range(4):
        nc.sync.dma_start(out=eb_sb[:, GW * h : GW * (h + 1)], in_=eb[h])
    x_sb = [[persist.tile([128, N], BF16, tag=f"x{b}_{cc}", name=f"x{b}_{cc}") for cc in range(2)] for b in range(B_LOC)]
    for cc in range(2):
        nc.sync.dma_start(out=x_sb[0][cc][:], in_=x[0, 128 * cc : 128 * (cc + 1), :])
    wq_sb, wk_sb, wv_sb, wo_sb = [], [], [], []
    for cc in range(2):
        for lst, src, nm in (
            (wq_sb, wqT, "wq"),
            (wk_sb, wkT, "wk"),
            (wv_sb, wvT, "wv"),
            (wo_sb, woT, "wo"),
        ):
            t = persist.tile([128, C], BF16, tag=f"{nm}{cc}", name=f"{nm}{cc}")
            nc.sync.dma_start(out=t[:], in_=src[128 * cc : 128 * (cc + 1), :])
            lst.append(t)
    bo_sb = []
    for cc in range(2):
        t = persist.tile([128, 1], F32, tag=f"bo{cc}", name=f"bo{cc}")
        nc.sync.dma_start(out=t[:], in_=bo[128 * cc : 128 * (cc + 1), :])
        bo_sb.append(t)
    for h in range(4, HEADS):
        nc.sync.dma_start(out=eb_sb[:, GW * h : GW * (h + 1)], in_=eb[h])
    for cc in range(2):
        nc.sync.dma_start(out=x_sb[1][cc][:], in_=x[1, 128 * cc : 128 * (cc + 1), :])
    ones32_sb = persist.tile([128, 32], BF16, tag="ones32", name="ones32")
    nc.vector.memset(ones32_sb[:], 1.0)
    # warm up the exp table set (~2.7us ACT_TABLE_LOAD) under the prologue
    warm = small.tile([1, 32], F32, tag="warm", name="warm_t")
    nc.scalar.activation(out=warm[:], in_=ones32_sb[0:1, :], func=AF.Exp)

    # ---------- persistent stage-A outputs ----------
    qT_sb = [[persist.tile([128, N], BF16, tag=f"qT{b}_{oc}", name=f"qT{b}_{oc}") for oc in range(2)] for b in range(B_LOC)]
    kT_sb = [[persist.tile([128, N], BF16, tag=f"kT{b}_{oc}", name=f"kT{b}_{oc}") for oc in range(2)] for b in range(B_LOC)]
    # v: [n, o] layout, 8 row tiles of 128 tokens, ones column per head
    # (33 cols/head) so P@V emits the softmax denominator via ones32 matmuls.
    v_sb = [[persist.tile([128, 33 * HEADS], BF16, tag=f"v{b}_{nt}", name=f"v{b}_{nt}") for nt in range(8)] for b in range(B_LOC)]
    otn_sb = [[persist.tile([128, N], BF16, tag=f"otn{b}_{ch}", name=f"otn{b}_{ch}") for ch in range(2)] for b in range(B_LOC)]

    def emit_qk_group(b, oc, nc2, w_sb, dst, pool_tile):
        for cc in range(2):
            nc.tensor.matmul(
                pool_tile[:, 0:512],
                lhsT=w_sb[cc][:, 128 * oc : 128 * (oc + 1)],
                rhs=x_sb[b][cc][:, 512 * nc2 : 512 * (nc2 + 1)],
                start=(cc == 0),
                stop=(cc == 1),
            )
        nc.vector.tensor_copy(
            out=dst[b][oc][:, 512 * nc2 : 512 * (nc2 + 1)], in_=pool_tile[:, 0:512]
        )

    def emit_v_group(b, nt, pool_tile):
        for cc in range(2):
            nc.tensor.matmul(
                pool_tile[:, 0:C],
                lhsT=x_sb[b][cc][:, 128 * nt : 128 * (nt + 1)],
                rhs=wv_sb[cc][:],
                start=(cc == 0),
                stop=(cc == 1),
            )
        v33 = v_sb[b][nt][:].rearrange("p (h w) -> p h w", w=33)
        nc.vector.tensor_copy(
            out=v33[:, :, 0:32], in_=pool_tile[:, 0:C].rearrange("p (h w) -> p h w", w=32)
        )
        nc.vector.memset(v33[:, :, 32:33], 1.0)

    def stage_c_group(b, ct, q2, pool_tile):
        for ch in range(2):
            nc.tensor.matmul(
                pool_tile[:, 0:512],
                lhsT=wo_sb[ch][:, 128 * ct : 128 * (ct + 1)],
                rhs=otn_sb[b][ch][:, 512 * q2 : 512 * (q2 + 1)],
                start=(ch == 0),
                stop=(ch == 1),
            )
        ob = outp.tile([128, 512], F32, tag="ob", name="ob_t")
        nc.vector.tensor_scalar_add(out=ob[:], in0=pool_tile[:, 0:512], scalar1=bo_sb[ct][:])
        nc.sync.dma_start(
            out=out[b, 128 * ct : 128 * (ct + 1), 512 * q2 : 512 * (q2 + 1)],
            in_=ob[:],
        )

    # ---------- prologue stage A ----------
    # b0's projections run as a dense PE burst at normal priority (they gate
    # the first strips and the burst warms the HAM).  b1's projections and V
    # tiles are emitted at LOW scheduler priority: the Tile scheduler slots
    # them into PE idle slivers during the early strips instead of ahead of
    # the critical score matmuls.
    from contextlib import contextmanager

    @contextmanager
    def lowprio(off):
        tc.cur_priority += off
        try:
            yield
        finally:
            tc.cur_priority -= off

    def drip_tile():
        return ps_drip.tile([128, 512], F32, tag="drip", name="drip_ps")

    pi = 0

    def prol_tile():
        nonlocal pi
        pi += 1
        if pi % 3 == 0:
            return drip_tile()
        return ps_ot.tile([128, 512], F32, tag="ot", name="ot_ps")

    for oc in range(2):
        for nc2 in range(2):
            for w_sb, dst in ((wq_sb, qT_sb), (wk_sb, kT_sb)):
                emit_qk_group(0, oc, nc2, w_sb, dst, prol_tile())
    for nt in range(8):
        emit_v_group(0, nt, prol_tile())
    with lowprio(400):
        for nt in range(8):
            emit_v_group(1, nt, prol_tile())
        for oc in range(2):
            for nc2 in range(2):
                for w_sb, dst in ((wq_sb, qT_sb), (wk_sb, kT_sb)):
                    emit_qk_group(1, oc, nc2, w_sb, dst, drip_tile())

    # stage-C drips: strip -> thunk.  C(b, ct, q2) needs otn[b][*][:, 512q2:]
    # from both quads; quad1 norms land ~4 strips into the following qi block.
    c_drips = {}
    for i, (ct,) in enumerate([(0,), (1,)]):
        c_drips[45 + i] = lambda ct=ct: stage_c_group(0, ct, 0, drip_tile())
        c_drips[53 + i] = lambda ct=ct: stage_c_group(0, ct, 1, drip_tile())
        c_drips[60 + i] = lambda ct=ct: stage_c_group(1, ct, 0, drip_tile())

    # ---------- softmax normalization (per qi block) ----------
    eb3 = eb_sb[:].rearrange("p (h w) -> p h w", w=GW)

    def _make_norm(ot_, den_, b_, quad_, qi_):
        state = {}

        def part1():
            # den rows 32h2 hold head h2's denominators (32 identical rows).
            # One SBUF->SBUF DMA gathers the 4 head rows and reshapes them to
            # [32, 64] so the iterative reciprocal (8 cyc/elem) uses 32 lanes.
            den_sb = small.tile([128, 512], F32, tag="den_sb", name="den_sb_t")
            nc.vector.tensor_copy(out=den_sb[:], in_=den_[:])
            rden32 = small.tile([32, 64], F32, tag="rden32", name="rden32_t")
            nc.sync.dma_start(
                out=rden32[:],
                in_=den_sb[:].rearrange("(h r) (p j) -> h r p j", r=32, j=64)[:, 0:1, :, :],
            )
            state["rden32"] = rden32

        def part2():
            rden32 = state["rden32"]
            nc.vector.reciprocal(out=rden32[:], in_=rden32[:])
            rden_dr = dram_pool.tile([4, 512], F32, tag="rden_dr", name="rden_dr")
            nc.sync.dma_start(
                out=rden_dr[:].rearrange("f (p j) -> (f p) j", j=64), in_=rden32[:]
            )
            state["rden_dr"] = rden_dr

        def part3():
            # broadcast per-head reciprocal rows to the [128, 512] layout
            # matching ot rows, then ONE normalize mul for all 4 heads.
            rden_dr = state["rden_dr"]
            rdb = small.tile([128, 512], F32, tag="rdb", name="rdb_t")
            for h2 in range(4):
                nc.sync.dma_start(
                    out=rdb[32 * h2 : 32 * (h2 + 1), :],
                    in_=rden_dr[h2 : h2 + 1, :].to_broadcast([32, 512]),
                )
            nc.vector.tensor_mul(
                out=otn_sb[b_][quad_][:, 512 * qi_ : 512 * (qi_ + 1)],
                in0=ot_[:],
                in1=rdb[:],
            )

        return [part1, part2, part3]

    # ---------- stage B: 64 strips, lag-2 software pipeline ----------
    def emit_pvden(args):
        ot_, den_, b_, quad_, qi_, kt_, se_, first, last = args
        for h2 in range(4):
            nc.tensor.matmul(
                ot_[32 * h2 : 32 * (h2 + 1), :],
                lhsT=v_sb[b_][kt_][:, 33 * (4 * quad_ + h2) : 33 * (4 * quad_ + h2) + 32],
                rhs=se_[:, 512 * h2 : 512 * (h2 + 1)],
                start=first,
                stop=last,
                tile_position=(0, 32 * h2),
                skip_group_check=True,
            )
        for h2 in range(4):
            nc.tensor.matmul(
                den_[32 * h2 : 32 * (h2 + 1), :],
                lhsT=ones32_sb[:],
                rhs=se_[:, 512 * h2 : 512 * (h2 + 1)],
                start=first,
                stop=last,
                tile_position=(0, 32 * h2),
                skip_group_check=True,
            )
        if last:
            norm_parts.extend(_make_norm(ot_, den_, b_, quad_, qi_))

    BLOCKS = [(0, 0), (0, 1), (1, 0), (1, 1)]  # (quad, b)
    pending = deque()
    norm_parts = deque()
    block_acc = {}  # (qi,) accumulators for the current block

    for s in range(64):
        quad, b = BLOCKS[s // 16]
        qi = (s // 8) % 2
        kt = s % 8
        # norm part for a completed qi block (emitted BEFORE this strip's
        # lag-2 PVden so the den bank WAR resolves without a stall)
        if norm_parts:
            norm_parts.popleft()()
        if kt == 0:
            block_acc = (
                ps_ot.tile([128, 512], F32, tag="ot", name="ot_ps"),
                ps_den.tile([128, 512], F32, tag="den", name="den_ps"),
            )
        ot_cur, den_cur = block_acc
        st_lo = ps_lo.tile([128, 1024], F32, tag="stlo", name="stlo_ps")
        st_hi = ps_hi.tile([128, 1024], F32, tag="sthi", name="sthi_ps")
        se = stexp_pool.tile([128, 2048], BF16, tag="se", name="se_t")
        for h2 in range(4):
            dst = st_lo if h2 < 2 else st_hi
            nc.tensor.matmul(
                dst[:, 512 * (h2 % 2) : 512 * (h2 % 2 + 1)],
                lhsT=kT_sb[b][quad][32 * h2 : 32 * (h2 + 1), 128 * kt : 128 * (kt + 1)],
                rhs=qT_sb[b][quad][32 * h2 : 32 * (h2 + 1), 512 * qi : 512 * (qi + 1)],
                start=True,
                stop=True,
                tile_position=(32 * h2, 0),
            )
        nc.scalar.activation(out=se[:, 0:1024], in_=st_lo[:], func=AF.Exp)
        nc.scalar.activation(out=se[:, 1024:2048], in_=st_hi[:], func=AF.Exp)
        off = 896 - 128 * kt + 512 * qi
        nc.vector.tensor_mul(
            out=se[:].rearrange("p (h q) -> p h q", h=4),
            in0=se[:].rearrange("p (h q) -> p h q", h=4),
            in1=eb3[:, 4 * quad : 4 * quad + 4, off : off + 512],
        )
        pending.append((ot_cur, den_cur, b, quad, qi, kt, se, kt == 0, kt == 7))
        if len(pending) > 2:
            emit_pvden(pending.popleft())
        if s in c_drips:
            with lowprio(150):
                c_drips[s]()

    # ---------- tail ----------
    while pending:
        emit_pvden(pending.popleft())
    while norm_parts:
        norm_parts.popleft()()
    for ct in range(2):
        stage_c_group(1, ct, 1, drip_tile())


def build():
    nc = bacc.Bacc("TRN2", target_bir_lowering=False, debug=False, num_devices=N_CORES)
    io = {
        "x": nc.dram_tensor("x", [B_LOC, C, N], BF16, kind="ExternalInput").ap(),
        "wqT": nc.dram_tensor("wqT", [C, C], BF16, kind="ExternalInput").ap(),
        "wkT": nc.dram_tensor("wkT", [C, C], BF16, kind="ExternalInput").ap(),
        "wvT": nc.dram_tensor("wvT", [C, C], BF16, kind="ExternalInput").ap(),
        "woT": nc.dram_tensor("woT", [C, C], BF16, kind="ExternalInput").ap(),
        "bo": nc.dram_tensor("bo", [C, 1], F32, kind="ExternalInput").ap(),
        "eb": nc.dram_tensor("eb", [HEADS, 128, GW], BF16, kind="ExternalInput").ap(),
        "out": nc.dram_tensor("out", [B_LOC, C, N], F32, kind="ExternalOutput").ap(),
    }
    with tile.TileContext(nc) as tc:
        _emit(tc, io)
    nc.compile()
    return nc


_CACHE: dict = {}


def _get_nc():
    if "nc" not in _CACHE:
        _CACHE["nc"] = build()
    return _CACHE["nc"]


def make_in_maps(x, Wq, Wk, Wv, Wo, bo, rel_bias, rel_idx=None):
    """Host-side sharding/layout prep. Returns per-core input maps."""
    import ml_dtypes

    bf16 = ml_dtypes.bfloat16
    x = np.asarray(x, np.float32)
    b, c, H, W = x.shape
    assert (b, c, H * W) == (B_LOC * N_CORES, C, N)
    xr = np.ascontiguousarray(x.reshape(b, c, N).astype(bf16))
    wqT = np.ascontiguousarray(np.asarray(Wq, np.float32).T.astype(bf16))
    wkT = np.ascontiguousarray((np.asarray(Wk, np.float32) * SCALE).T.astype(bf16))
    wvT = np.ascontiguousarray(np.asarray(Wv, np.float32).T.astype(bf16))
    woT = np.ascontiguousarray(np.asarray(Wo, np.float32).T.astype(bf16))
    bo2 = np.ascontiguousarray(np.asarray(bo, np.float32).reshape(C, 1))
    rb = np.asarray(rel_bias, np.float32)
    idx = G0 + np.arange(128)[:, None] - np.arange(GW)[None, :]
    ebmat = np.ascontiguousarray(np.exp(rb[:, idx]).astype(bf16))  # [8, 128, GW]
    shared = dict(wqT=wqT, wkT=wkT, wvT=wvT, woT=woT, bo=bo2, eb=ebmat)
    return [
        dict(x=np.ascontiguousarray(xr[B_LOC * i : B_LOC * (i + 1)]), **shared)
        for i in range(N_CORES)
    ]


def _install_ntff_hook_shim():
    """bass_utils fetches the axon NTFF hook via antenv.axon_hooks, which this
    image's antenv lacks; synthesize it from trn_agent_boot's ctypes hook."""
    import sys
    import types

    try:
        from antenv.axon_hooks import get_axon_ntff_profile_hook  # noqa: F401

        return
    except ImportError:
        pass
    hook = None
    try:
        from trn_agent_boot.trn_boot import _ntff_profile_via_ctypes

        hook = _ntff_profile_via_ctypes("/opt/axon/libaxon_pjrt.so")
    except Exception:
        pass
    mod = types.ModuleType("antenv.axon_hooks")
    state = {"hook": hook}
    mod.get_axon_ntff_profile_hook = lambda: state["hook"]
    mod.set_axon_ntff_profile_hook = lambda h: state.__setitem__("hook", h)
    sys.modules["antenv.axon_hooks"] = mod


def run(inputs: dict, trace: bool = False):
    """Run on the 8 cores; returns (full_output, BassKernelResults)."""
    if trace:
        _install_ntff_hook_shim()
    in_maps = make_in_maps(**inputs)
    nc = _get_nc()
    res = bass_utils.run_bass_kernel_spmd(
        nc, in_maps, core_ids=list(range(N_CORES)), trace=trace
    )
    outs = np.stack([res.results[i]["out"] for i in range(N_CORES)])
    out = outs.reshape(B_LOC * N_CORES, C, 32, 32)
    return out, res


def kernel(**inputs) -> np.ndarray:
    out, _ = run(inputs)
    return out


# revision 9
# speedup vs baseline: 1.1865x; 1.0456x over previous
"""CoAtNet relative attention kernel for Trainium2 (Bass/Tile), 8 NeuronCores.

Problem (per full input):
  x [16, 256, 32, 32] f32; Wq/Wk/Wv [256, 256]; Wo [256, 256]; bo [256];
  rel_bias [8, 3969]; rel_idx [1024, 1024] int32 (static pattern).
  out[b] = softmax(q k^T / sqrt(d) + bias) v  projected back, heads=8, d=32.

Sharding: data-parallel over batch — each of the 8 cores handles 2 batches
with identical programs (SPMD, no collectives).

Key structural facts used:
  * rel_idx[p, q] == (q - p) + 1056 exactly (the reference's quirky *W stride
    collapses the 2D relative index to 1D Toeplitz).  So the [1024, 1024]
    bias matrix per head is bias[p, q] = rel_bias[h, q - p + 1056] and any
    [128, width] tile of it (keys on partitions) is a contiguous slice of a
    small "sheared" tile  G[h, i, j'] = rel_bias[h, 1952 + i - j']  of shape
    [128, 1920].  No gather on device at all.  The bias is applied as
    exp(S+B) = exp(S) * exp(B) with exp(B) precomputed, so the application
    is a bf16 2x-mode multiply instead of an fp32 1x add.
  * Everything is computed in "transposed" layout so no transposes are ever
    needed: x arrives as [c, n] per batch; Q^T/K^T = W @ x are [d_all, n];
    scores are built as S^T [keys, queries]; P@V uses lhsT = V directly;
    and the final projection produces out^T [c, n], exactly the output
    memory layout.
  * The kernel is ACT(exp)-throughput-bound: 16.8M exps/core at 1 elem/
    lane/cycle @1.2GHz is a ~110us floor.  The schedule is built so the
    ScalarE never waits: each strip's exp is SPLIT into two ACTIVATEs over
    separate PSUM tiles (st_lo banks 0-1, st_hi banks 2-3) so the next
    strip's score matmuls can overwrite the low banks while ACT still
    processes the high banks (the single-buffered [128,2048] fp32 score
    tile cannot be double-buffered: TRN2 matmuls write fp32-only and PSUM
    has just 8 banks).  PV/den matmuls run TWO strips behind the score
    matmuls so a DVE-gated PV never head-of-line blocks ready ST work in
    the PE FIFO.  All projection / output-projection / softmax-
    normalization work is dripped into per-strip slack slots on the other
    engines.
"""

import numpy as np
from collections import deque
from contextlib import ExitStack

import concourse.bass as bass
import concourse.bacc as bacc
import concourse.mybir as mybir
import concourse.tile as tile
from concourse import bass_utils
from concourse._compat import with_exitstack

HEADS = 8
D = 32  # head dim
C = 256  # channels = heads * D
N = 1024  # tokens = 32 * 32
B_LOC = 2  # batches per core
N_CORES = 8
SCALE = D ** -0.5
GW = 1920  # sheared bias tile width
G0 = 1952  # G[h, i, j'] = rel_bias[h, G0 + i - j']

F32 = mybir.dt.float32
BF16 = mybir.dt.bfloat16
AF = mybir.ActivationFunctionType


@with_exitstack
def _emit(ctx: ExitStack, tc: tile.TileContext, io: dict):
    nc = tc.nc
    x, wqT, wkT, wvT, woT, bo, eb, out = (
        io[k] for k in ("x", "wqT", "wkT", "wvT", "woT", "bo", "eb", "out")
    )

    persist = ctx.enter_context(tc.tile_pool(name="persist", bufs=1))
    stexp_pool = ctx.enter_context(tc.tile_pool(name="stexp", bufs=4))
    small = ctx.enter_context(tc.tile_pool(name="small", bufs=2))
    outp = ctx.enter_context(tc.tile_pool(name="outp", bufs=4))
    # PSUM budget (8 banks): st_lo 2 + st_hi 2 + ot 2x1 + den 1x1 + drip 1.
    ps_lo = ctx.enter_context(tc.tile_pool(name="ps_lo", bufs=1, space="PSUM"))
    ps_hi = ctx.enter_context(tc.tile_pool(name="ps_hi", bufs=1, space="PSUM"))
    ps_ot = ctx.enter_context(tc.tile_pool(name="ps_ot", bufs=2, space="PSUM"))
    ps_den = ctx.enter_context(tc.tile_pool(name="ps_den", bufs=1, space="PSUM"))
    ps_drip = ctx.enter_context(tc.tile_pool(name="ps_drip", bufs=1, space="PSUM"))

    # ---------- DMAs: everything in flight up front ----------
    # eb quad0 slices first (needed by the very first bias multiply).
    eb_sb = persist.tile([128, HEADS * GW], BF16, tag="eb", name="eb_sb")
    for h in range(4):
        nc.sync.dma_start(out=eb_sb[:, GW * h : GW * (h + 1)], in_=eb[h])
    x_sb = [[persist.tile([128, N], BF16, tag=f"x{b}_{cc}", name=f"x{b}_{cc}") for cc in range(2)] for b in range(B_LOC)]
    for cc in range(2):
        nc.sync.dma_start(out=x_sb[0][cc][:], in_=x[0, 128 * cc : 128 * (cc + 1), :])
    wq_sb, wk_sb, wv_sb, wo_sb = [], [], [], []
    for cc in range(2):
        for lst, src, nm in (
            (wq_sb, wqT, "wq"),
            (wk_sb, wkT, "wk"),
            (wv_sb, wvT, "wv"),
            (wo_sb, woT, "wo"),
        ):
            t = persist.tile([128, C], BF16, tag=f"{nm}{cc}", name=f"{nm}{cc}")
            nc.sync.dma_start(out=t[:], in_=src[128 * cc : 128 * (cc + 1), :])
            lst.append(t)
    bo_sb = []
    for cc in range(2):
        t = persist.tile([128, 1], F32, tag=f"bo{cc}", name=f"bo{cc}")
        nc.sync.dma_start(out=t[:], in_=bo[128 * cc : 128 * (cc + 1), :])
        bo_sb.append(t)
    for h in range(4, HEADS):
        nc.sync.dma_start(out=eb_sb[:, GW * h : GW * (h + 1)], in_=eb[h])
    for cc in range(2):
        nc.sync.dma_start(out=x_sb[1][cc][:], in_=x[1, 128 * cc : 128 * (cc + 1), :])
    ones32_sb = persist.tile([128, 32], BF16, tag="ones32", name="ones32")
    nc.vector.memset(ones32_sb[:], 1.0)
    # warm up the exp table set (~2.7us ACT_TABLE_LOAD) under the prologue
    warm = small.tile([1, 32], F32, tag="warm", name="warm_t")
    nc.scalar.activation(out=warm[:], in_=ones32_sb[0:1, :], func=AF.Exp)

    # ---------- persistent stage-A outputs ----------
    qT_sb = [[persist.tile([128, N], BF16, tag=f"qT{b}_{oc}", name=f"qT{b}_{oc}") for oc in range(2)] for b in range(B_LOC)]
    kT_sb = [[persist.tile([128, N], BF16, tag=f"kT{b}_{oc}", name=f"kT{b}_{oc}") for oc in range(2)] for b in range(B_LOC)]
    # v: [n, o] layout, 8 row tiles of 128 tokens, ones column per head
    # (33 cols/head) so P@V emits the softmax denominator via ones32 matmuls.
    v_sb = [[persist.tile([128, 33 * HEADS], BF16, tag=f"v{b}_{nt}", name=f"v{b}_{nt}") for nt in range(8)] for b in range(B_LOC)]
    otn_sb = [[persist.tile([128, N], BF16, tag=f"otn{b}_{ch}", name=f"otn{b}_{ch}") for ch in range(2)] for b in range(B_LOC)]

    def emit_qk_group(b, oc, nc2, w_sb, dst, pool_tile):
        for cc in range(2):
            nc.tensor.matmul(
                pool_tile[:, 0:512],
                lhsT=w_sb[cc][:, 128 * oc : 128 * (oc + 1)],
                rhs=x_sb[b][cc][:, 512 * nc2 : 512 * (nc2 + 1)],
                start=(cc == 0),
                stop=(cc == 1),
            )
        nc.vector.tensor_copy(
            out=dst[b][oc][:, 512 * nc2 : 512 * (nc2 + 1)], in_=pool_tile[:, 0:512]
        )

    def emit_v_group(b, nt, pool_tile):
        for cc in range(2):
            nc.tensor.matmul(
                pool_tile[:, 0:C],
                lhsT=x_sb[b][cc][:, 128 * nt : 128 * (nt + 1)],
                rhs=wv_sb[cc][:],
                start=(cc == 0),
                stop=(cc == 1),
            )
        v33 = v_sb[b][nt][:].rearrange("p (h w) -> p h w", w=33)
        nc.vector.tensor_copy(
            out=v33[:, :, 0:32], in_=pool_tile[:, 0:C].rearrange("p (h w) -> p h w", w=32)
        )
        nc.vector.memset(v33[:, :, 32:33], 1.0)

    def stage_c_group(b, ct, q2, pool_tile):
        for ch in range(2):
            nc.tensor.matmul(
                pool_tile[:, 0:512],
                lhsT=wo_sb[ch][:, 128 * ct : 128 * (ct + 1)],
                rhs=otn_sb[b][ch][:, 512 * q2 : 512 * (q2 + 1)],
                start=(ch == 0),
                stop=(ch == 1),
            )
        ob = outp.tile([128, 512], F32, tag="ob", name="ob_t")
        nc.vector.tensor_scalar_add(out=ob[:], in0=pool_tile[:, 0:512], scalar1=bo_sb[ct][:])
        nc.sync.dma_start(
            out=out[b, 128 * ct : 128 * (ct + 1), 512 * q2 : 512 * (q2 + 1)],
            in_=ob[:],
        )

    # ---------- prologue stage A ----------
    # b0's projections run as a dense PE burst at normal priority (they gate
    # the first strips and the burst warms the HAM).  b1's projections and V
    # tiles are emitted at LOW scheduler priority: the Tile scheduler slots
    # them into PE idle slivers during the early strips instead of ahead of
    # the critical score matmuls.
    from contextlib import contextmanager

    @contextmanager
    def lowprio(off):
        tc.cur_priority += off
        try:
            yield
        finally:
            tc.cur_priority -= off

    def drip_tile():
        return ps_drip.tile([128, 512], F32, tag="drip", name="drip_ps")

    pi = 0

    def prol_tile():
        nonlocal pi
        pi += 1
        if pi % 3 == 0:
            return drip_tile()
        return ps_ot.tile([128, 512], F32, tag="ot", name="ot_ps")

    for oc in range(2):
        for nc2 in range(2):
            for w_sb, dst in ((wq_sb, qT_sb), (wk_sb, kT_sb)):
                emit_qk_group(0, oc, nc2, w_sb, dst, prol_tile())
    for nt in range(8):
        emit_v_group(0, nt, prol_tile())
    with lowprio(400):
        for nt in range(8):
            emit_v_group(1, nt, prol_tile())
        for oc in range(2):
            for nc2 in range(2):
                for w_sb, dst in ((wq_sb, qT_sb), (wk_sb, kT_sb)):
                    emit_qk_group(1, oc, nc2, w_sb, dst, drip_tile())

    # stage-C drips: strip -> thunk.  C(b, ct, q2) needs otn[b][*][:, 512q2:]
    # from both quads; quad1 norms land ~4 strips into the following qi block.
    c_drips = {}
    for i, (ct,) in enumerate([(0,), (1,)]):
        c_drips[45 + i] = lambda ct=ct: stage_c_group(0, ct, 0, drip_tile())
        c_drips[53 + i] = lambda ct=ct: stage_c_group(0, ct, 1, drip_tile())
        c_drips[60 + i] = lambda ct=ct: stage_c_group(1, ct, 0, drip_tile())

    # ---------- softmax normalization (per qi block) ----------
    eb3 = eb_sb[:].rearrange("p (h w) -> p h w", w=GW)

    def _make_norm(ot_, den_, b_, quad_, qi_):
        # The den accumulator's rows 32h2..32h2+31 hold 32 identical copies
        # of head h2's denominators (M=32 col-tiled ones matmul), i.e. the
        # tile is ALREADY in row-broadcast layout.  The whole normalization
        # is two DVE ops: approx-reciprocal (51 ULP, plenty under the 2e-2
        # budget; denominators are benign positive sums) and one multiply.
        state = {}

        def part1():
            rdb = small.tile([128, 512], F32, tag="rdb", name="rdb_t")
            nc.vector.reciprocal_approx_fast(out=rdb[:], in_=den_[:])
            state["rdb"] = rdb

        def part2():
            nc.vector.tensor_mul(
                out=otn_sb[b_][quad_][:, 512 * qi_ : 512 * (qi_ + 1)],
                in0=ot_[:],
                in1=state["rdb"][:],
            )

        return [part1, part2]

    # ---------- stage B: 64 strips, lag-2 software pipeline ----------
    def emit_pvden(args):
        ot_, den_, b_, quad_, qi_, kt_, se_, first, last = args
        for h2 in range(4):
            nc.tensor.matmul(
                ot_[32 * h2 : 32 * (h2 + 1), :],
                lhsT=v_sb[b_][kt_][:, 33 * (4 * quad_ + h2) : 33 * (4 * quad_ + h2) + 32],
                rhs=se_[:, 512 * h2 : 512 * (h2 + 1)],
                start=first,
                stop=last,
                tile_position=(0, 32 * h2),
                skip_group_check=True,
            )
        for h2 in range(4):
            nc.tensor.matmul(
                den_[32 * h2 : 32 * (h2 + 1), :],
                lhsT=ones32_sb[:],
                rhs=se_[:, 512 * h2 : 512 * (h2 + 1)],
                start=first,
                stop=last,
                tile_position=(0, 32 * h2),
                skip_group_check=True,
            )
        if last:
            norm_parts.extend(_make_norm(ot_, den_, b_, quad_, qi_))

    BLOCKS = [(0, 0), (0, 1), (1, 0), (1, 1)]  # (quad, b)
    pending = deque()
    norm_parts = deque()
    block_acc = {}  # (qi,) accumulators for the current block

    for s in range(64):
        quad, b = BLOCKS[s // 16]
        qi = (s // 8) % 2
        kt = s % 8
        # norm part for a completed qi block (emitted BEFORE this strip's
        # lag-2 PVden so the den bank WAR resolves without a stall)
        if norm_parts:
            norm_parts.popleft()()
        if kt == 0:
            block_acc = (
                ps_ot.tile([128, 512], F32, tag="ot", name="ot_ps"),
                ps_den.tile([128, 512], F32, tag="den", name="den_ps"),
            )
        ot_cur, den_cur = block_acc
        st_lo = ps_lo.tile([128, 1024], F32, tag="stlo", name="stlo_ps")
        st_hi = ps_hi.tile([128, 1024], F32, tag="sthi", name="sthi_ps")
        se = stexp_pool.tile([128, 2048], BF16, tag="se", name="se_t")
        for h2 in range(4):
            dst = st_lo if h2 < 2 else st_hi
            nc.tensor.matmul(
                dst[:, 512 * (h2 % 2) : 512 * (h2 % 2 + 1)],
                lhsT=kT_sb[b][quad][32 * h2 : 32 * (h2 + 1), 128 * kt : 128 * (kt + 1)],
                rhs=qT_sb[b][quad][32 * h2 : 32 * (h2 + 1), 512 * qi : 512 * (qi + 1)],
                start=True,
                stop=True,
                tile_position=(32 * h2, 0),
            )
        nc.scalar.activation(out=se[:, 0:1024], in_=st_lo[:], func=AF.Exp)
        nc.scalar.activation(out=se[:, 1024:2048], in_=st_hi[:], func=AF.Exp)
        off = 896 - 128 * kt + 512 * qi
        nc.vector.tensor_mul(
            out=se[:].rearrange("p (h q) -> p h q", h=4),
            in0=se[:].rearrange("p (h q) -> p h q", h=4),
            in1=eb3[:, 4 * quad : 4 * quad + 4, off : off + 512],
        )
        pending.append((ot_cur, den_cur, b, quad, qi, kt, se, kt == 0, kt == 7))
        if len(pending) > 2:
            emit_pvden(pending.popleft())
        if s in c_drips:
            with lowprio(150):
                c_drips[s]()

    # ---------- tail ----------
    while pending:
        emit_pvden(pending.popleft())
    while norm_parts:
        norm_parts.popleft()()
    for ct in range(2):
        stage_c_group(1, ct, 1, drip_tile())


def build():
    nc = bacc.Bacc("TRN2", target_bir_lowering=False, debug=False, num_devices=N_CORES)
    io = {
        "x": nc.dram_tensor("x", [B_LOC, C, N], BF16, kind="ExternalInput").ap(),
        "wqT": nc.dram_tensor("wqT", [C, C], BF16, kind="ExternalInput").ap(),
        "wkT": nc.dram_tensor("wkT", [C, C], BF16, kind="ExternalInput").ap(),
        "wvT": nc.dram_tensor("wvT", [C, C], BF16, kind="ExternalInput").ap(),
        "woT": nc.dram_tensor("woT", [C, C], BF16, kind="ExternalInput").ap(),
        "bo": nc.dram_tensor("bo", [C, 1], F32, kind="ExternalInput").ap(),
        "eb": nc.dram_tensor("eb", [HEADS, 128, GW], BF16, kind="ExternalInput").ap(),
        "out": nc.dram_tensor("out", [B_LOC, C, N], F32, kind="ExternalOutput").ap(),
    }
    with tile.TileContext(nc) as tc:
        _emit(tc, io)
    nc.compile()
    return nc


_CACHE: dict = {}


def _get_nc():
    if "nc" not in _CACHE:
        _CACHE["nc"] = build()
    return _CACHE["nc"]


def make_in_maps(x, Wq, Wk, Wv, Wo, bo, rel_bias, rel_idx=None):
    """Host-side sharding/layout prep. Returns per-core input maps."""
    import ml_dtypes

    bf16 = ml_dtypes.bfloat16
    x = np.asarray(x, np.float32)
    b, c, H, W = x.shape
    assert (b, c, H * W) == (B_LOC * N_CORES, C, N)
    xr = np.ascontiguousarray(x.reshape(b, c, N).astype(bf16))
    wqT = np.ascontiguousarray(np.asarray(Wq, np.float32).T.astype(bf16))
    wkT = np.ascontiguousarray((np.asarray(Wk, np.float32) * SCALE).T.astype(bf16))
    wvT = np.ascontiguousarray(np.asarray(Wv, np.float32).T.astype(bf16))
    woT = np.ascontiguousarray(np.asarray(Wo, np.float32).T.astype(bf16))
    bo2 = np.ascontiguousarray(np.asarray(bo, np.float32).reshape(C, 1))
    rb = np.asarray(rel_bias, np.float32)
    idx = G0 + np.arange(128)[:, None] - np.arange(GW)[None, :]
    ebmat = np.ascontiguousarray(np.exp(rb[:, idx]).astype(bf16))  # [8, 128, GW]
    shared = dict(wqT=wqT, wkT=wkT, wvT=wvT, woT=woT, bo=bo2, eb=ebmat)
    return [
        dict(x=np.ascontiguousarray(xr[B_LOC * i : B_LOC * (i + 1)]), **shared)
        for i in range(N_CORES)
    ]


def _install_ntff_hook_shim():
    """bass_utils fetches the axon NTFF hook via antenv.axon_hooks, which this
    image's antenv lacks; synthesize it from trn_agent_boot's ctypes hook."""
    import sys
    import types

    try:
        from antenv.axon_hooks import get_axon_ntff_profile_hook  # noqa: F401

        return
    except ImportError:
        pass
    hook = None
    try:
        from trn_agent_boot.trn_boot import _ntff_profile_via_ctypes

        hook = _ntff_profile_via_ctypes("/opt/axon/libaxon_pjrt.so")
    except Exception:
        pass
    mod = types.ModuleType("antenv.axon_hooks")
    state = {"hook": hook}
    mod.get_axon_ntff_profile_hook = lambda: state["hook"]
    mod.set_axon_ntff_profile_hook = lambda h: state.__setitem__("hook", h)
    sys.modules["antenv.axon_hooks"] = mod


def run(inputs: dict, trace: bool = False):
    """Run on the 8 cores; returns (full_output, BassKernelResults)."""
    if trace:
        _install_ntff_hook_shim()
    in_maps = make_in_maps(**inputs)
    nc = _get_nc()
    res = bass_utils.run_bass_kernel_spmd(
        nc, in_maps, core_ids=list(range(N_CORES)), trace=trace
    )
    outs = np.stack([res.results[i]["out"] for i in range(N_CORES)])
    out = outs.reshape(B_LOC * N_CORES, C, 32, 32)
    return out, res


def kernel(**inputs) -> np.ndarray:
    out, _ = run(inputs)
    return out


# revision 11
# speedup vs baseline: 1.2100x; 1.0198x over previous
"""CoAtNet relative attention kernel for Trainium2 (Bass/Tile), 8 NeuronCores.

Problem (per full input):
  x [16, 256, 32, 32] f32; Wq/Wk/Wv [256, 256]; Wo [256, 256]; bo [256];
  rel_bias [8, 3969]; rel_idx [1024, 1024] int32 (static pattern).
  out[b] = softmax(q k^T / sqrt(d) + bias) v  projected back, heads=8, d=32.

Sharding: data-parallel over batch — each of the 8 cores handles 2 batches
with identical programs (SPMD, no collectives).

Key structural facts used:
  * rel_idx[p, q] == (q - p) + 1056 exactly (the reference's quirky *W stride
    collapses the 2D relative index to 1D Toeplitz).  So the [1024, 1024]
    bias matrix per head is bias[p, q] = rel_bias[h, q - p + 1056] and any
    [128, width] tile of it (keys on partitions) is a contiguous slice of a
    small "sheared" tile  G[h, i, j'] = rel_bias[h, 1952 + i - j']  of shape
    [128, 1920].  No gather on device at all.  The bias is applied as
    exp(S+B) = exp(S) * exp(B) with exp(B) precomputed, so the application
    is a bf16 2x-mode multiply instead of an fp32 1x add.
  * Everything is computed in "transposed" layout so no transposes are ever
    needed: x arrives as [c, n] per batch; Q^T/K^T = W @ x are [d_all, n];
    scores are built as S^T [keys, queries]; P@V uses lhsT = V directly;
    and the final projection produces out^T [c, n], exactly the output
    memory layout.
  * The kernel is ACT(exp)-throughput-bound: 16.8M exps/core at 1 elem/
    lane/cycle @1.2GHz is a ~110us floor.  The schedule is built so the
    ScalarE never waits: each strip's exp is SPLIT into two ACTIVATEs over
    separate PSUM tiles (st_lo banks 0-1, st_hi banks 2-3) so the next
    strip's score matmuls can overwrite the low banks while ACT still
    processes the high banks (the single-buffered [128,2048] fp32 score
    tile cannot be double-buffered: TRN2 matmuls write fp32-only and PSUM
    has just 8 banks).  PV/den matmuls run TWO strips behind the score
    matmuls so a DVE-gated PV never head-of-line blocks ready ST work in
    the PE FIFO.  All projection / output-projection / softmax-
    normalization work is dripped into per-strip slack slots on the other
    engines.
"""

import numpy as np
from collections import deque
from contextlib import ExitStack

import concourse.bass as bass
import concourse.bacc as bacc
import concourse.mybir as mybir
import concourse.tile as tile
from concourse import bass_utils
from concourse._compat import with_exitstack

HEADS = 8
D = 32  # head dim
C = 256  # channels = heads * D
N = 1024  # tokens = 32 * 32
B_LOC = 2  # batches per core
N_CORES = 8
SCALE = D ** -0.5
GW = 1920  # sheared bias tile width
G0 = 1952  # G[h, i, j'] = rel_bias[h, G0 + i - j']

F32 = mybir.dt.float32
BF16 = mybir.dt.bfloat16
AF = mybir.ActivationFunctionType


@with_exitstack
def _emit(ctx: ExitStack, tc: tile.TileContext, io: dict):
    nc = tc.nc
    x, wqT, wkT, wvT, woT, bo, eb, out = (
        io[k] for k in ("x", "wqT", "wkT", "wvT", "woT", "bo", "eb", "out")
    )

    persist = ctx.enter_context(tc.tile_pool(name="persist", bufs=1))
    stexp_pool = ctx.enter_context(tc.tile_pool(name="stexp", bufs=4))
    small = ctx.enter_context(tc.tile_pool(name="small", bufs=2))
    outp = ctx.enter_context(tc.tile_pool(name="outp", bufs=4))
    # PSUM budget (8 banks): st_lo 2 + st_hi 2 + ot 2x1 + den 1x1 + drip 1.
    ps_lo = ctx.enter_context(tc.tile_pool(name="ps_lo", bufs=1, space="PSUM"))
    ps_hi = ctx.enter_context(tc.tile_pool(name="ps_hi", bufs=1, space="PSUM"))
    ps_ot = ctx.enter_context(tc.tile_pool(name="ps_ot", bufs=2, space="PSUM"))
    ps_den = ctx.enter_context(tc.tile_pool(name="ps_den", bufs=1, space="PSUM"))
    ps_drip = ctx.enter_context(tc.tile_pool(name="ps_drip", bufs=1, space="PSUM"))

    # ---------- DMAs: everything in flight up front ----------
    # eb quad0 slices first (needed by the very first bias multiply).
    eb_sb = persist.tile([128, HEADS * GW], BF16, tag="eb", name="eb_sb")
    for h in range(4):
        nc.sync.dma_start(out=eb_sb[:, GW * h : GW * (h + 1)], in_=eb[h])
    x_sb = [[persist.tile([128, N], BF16, tag=f"x{b}_{cc}", name=f"x{b}_{cc}") for cc in range(2)] for b in range(B_LOC)]
    for cc in range(2):
        nc.sync.dma_start(out=x_sb[0][cc][:], in_=x[0, 128 * cc : 128 * (cc + 1), :])
    wq_sb, wk_sb, wv_sb, wo_sb = [], [], [], []
    for cc in range(2):
        for lst, src, nm in (
            (wq_sb, wqT, "wq"),
            (wk_sb, wkT, "wk"),
            (wv_sb, wvT, "wv"),
            (wo_sb, woT, "wo"),
        ):
            t = persist.tile([128, C], BF16, tag=f"{nm}{cc}", name=f"{nm}{cc}")
            nc.sync.dma_start(out=t[:], in_=src[128 * cc : 128 * (cc + 1), :])
            lst.append(t)
    bo_sb = []
    for cc in range(2):
        t = persist.tile([128, 1], F32, tag=f"bo{cc}", name=f"bo{cc}")
        nc.sync.dma_start(out=t[:], in_=bo[128 * cc : 128 * (cc + 1), :])
        bo_sb.append(t)
    for h in range(4, HEADS):
        nc.sync.dma_start(out=eb_sb[:, GW * h : GW * (h + 1)], in_=eb[h])
    for cc in range(2):
        nc.sync.dma_start(out=x_sb[1][cc][:], in_=x[1, 128 * cc : 128 * (cc + 1), :])
    ones32_sb = persist.tile([128, 32], BF16, tag="ones32", name="ones32")
    nc.vector.memset(ones32_sb[:], 1.0)
    # warm up the exp table set (~2.7us ACT_TABLE_LOAD) under the prologue
    warm = small.tile([1, 32], F32, tag="warm", name="warm_t")
    nc.scalar.activation(out=warm[:], in_=ones32_sb[0:1, :], func=AF.Exp)

    # ---------- persistent stage-A outputs ----------
    qT_sb = [[persist.tile([128, N], BF16, tag=f"qT{b}_{oc}", name=f"qT{b}_{oc}") for oc in range(2)] for b in range(B_LOC)]
    kT_sb = [[persist.tile([128, N], BF16, tag=f"kT{b}_{oc}", name=f"kT{b}_{oc}") for oc in range(2)] for b in range(B_LOC)]
    # v: [n, o] layout, 8 row tiles of 128 tokens, ones column per head
    # (33 cols/head) so P@V emits the softmax denominator via ones32 matmuls.
    v_sb = [[persist.tile([128, 33 * HEADS], BF16, tag=f"v{b}_{nt}", name=f"v{b}_{nt}") for nt in range(8)] for b in range(B_LOC)]
    otn_sb = [[persist.tile([128, N], BF16, tag=f"otn{b}_{ch}", name=f"otn{b}_{ch}") for ch in range(2)] for b in range(B_LOC)]

    def emit_qk_group(b, oc, nc2, w_sb, dst, pool_tile):
        for cc in range(2):
            nc.tensor.matmul(
                pool_tile[:, 0:512],
                lhsT=w_sb[cc][:, 128 * oc : 128 * (oc + 1)],
                rhs=x_sb[b][cc][:, 512 * nc2 : 512 * (nc2 + 1)],
                start=(cc == 0),
                stop=(cc == 1),
            )
        nc.vector.tensor_copy(
            out=dst[b][oc][:, 512 * nc2 : 512 * (nc2 + 1)], in_=pool_tile[:, 0:512]
        )

    def emit_v_group(b, nt, pool_tile):
        for cc in range(2):
            nc.tensor.matmul(
                pool_tile[:, 0:C],
                lhsT=x_sb[b][cc][:, 128 * nt : 128 * (nt + 1)],
                rhs=wv_sb[cc][:],
                start=(cc == 0),
                stop=(cc == 1),
            )
        v33 = v_sb[b][nt][:].rearrange("p (h w) -> p h w", w=33)
        nc.vector.tensor_copy(
            out=v33[:, :, 0:32], in_=pool_tile[:, 0:C].rearrange("p (h w) -> p h w", w=32)
        )
        nc.vector.memset(v33[:, :, 32:33], 1.0)

    def stage_c_group(b, ct, q2, pool_tile):
        for ch in range(2):
            nc.tensor.matmul(
                pool_tile[:, 0:512],
                lhsT=wo_sb[ch][:, 128 * ct : 128 * (ct + 1)],
                rhs=otn_sb[b][ch][:, 512 * q2 : 512 * (q2 + 1)],
                start=(ch == 0),
                stop=(ch == 1),
            )
        ob = outp.tile([128, 512], F32, tag="ob", name="ob_t")
        nc.vector.tensor_scalar_add(out=ob[:], in0=pool_tile[:, 0:512], scalar1=bo_sb[ct][:])
        nc.sync.dma_start(
            out=out[b, 128 * ct : 128 * (ct + 1), 512 * q2 : 512 * (q2 + 1)],
            in_=ob[:],
        )

    # ---------- prologue stage A ----------
    # b0's projections run as a dense PE burst at normal priority (they gate
    # the first strips and the burst warms the HAM).  b1's projections and V
    # tiles are emitted at LOW scheduler priority: the Tile scheduler slots
    # them into PE idle slivers during the early strips instead of ahead of
    # the critical score matmuls.
    from contextlib import contextmanager

    @contextmanager
    def lowprio(off):
        tc.cur_priority += off
        try:
            yield
        finally:
            tc.cur_priority -= off

    def drip_tile():
        return ps_drip.tile([128, 512], F32, tag="drip", name="drip_ps")

    pi = 0

    def prol_tile():
        nonlocal pi
        pi += 1
        if pi % 3 == 0:
            return drip_tile()
        return ps_ot.tile([128, 512], F32, tag="ot", name="ot_ps")

    # phase 1 (normal priority, dense): exactly what the first qi blocks
    # need — b0/quad0 q,k and all of b0's V (PV consumes v[kt] from strip 2).
    for nc2 in range(2):
        for w_sb, dst in ((wq_sb, qT_sb), (wk_sb, kT_sb)):
            emit_qk_group(0, 0, nc2, w_sb, dst, prol_tile())
    for nt in range(8):
        emit_v_group(0, nt, prol_tile())
    # phase 2 (low priority, deadline order, own PSUM bank so the critical
    # qi accumulators never inherit a deprioritized group's bank WAR):
    # qk(b1,oc0) by strip 16, v(b1) by strip 18+, qk(b0,oc1) by strip 32,
    # qk(b1,oc1) by strip 48.
    with lowprio(150):
        for nc2 in range(2):
            for w_sb, dst in ((wq_sb, qT_sb), (wk_sb, kT_sb)):
                emit_qk_group(1, 0, nc2, w_sb, dst, drip_tile())
        for nt in range(8):
            emit_v_group(1, nt, drip_tile())
        for b, oc in ((0, 1), (1, 1)):
            for nc2 in range(2):
                for w_sb, dst in ((wq_sb, qT_sb), (wk_sb, kT_sb)):
                    emit_qk_group(b, oc, nc2, w_sb, dst, drip_tile())

    # stage-C drips: strip -> thunk.  C(b, ct, q2) needs otn[b][*][:, 512q2:]
    # from both quads; quad1 norms land ~4 strips into the following qi block.
    c_drips = {}
    for i, (ct,) in enumerate([(0,), (1,)]):
        c_drips[45 + i] = lambda ct=ct: stage_c_group(0, ct, 0, drip_tile())
        c_drips[53 + i] = lambda ct=ct: stage_c_group(0, ct, 1, drip_tile())
        c_drips[60 + i] = lambda ct=ct: stage_c_group(1, ct, 0, drip_tile())

    # ---------- softmax normalization (per qi block) ----------
    eb3 = eb_sb[:].rearrange("p (h w) -> p h w", w=GW)

    def _make_norm(ot_, den_, b_, quad_, qi_):
        # The den accumulator's rows 32h2..32h2+31 hold 32 identical copies
        # of head h2's denominators (M=32 col-tiled ones matmul), i.e. the
        # tile is ALREADY in row-broadcast layout.  The whole normalization
        # is two DVE ops: approx-reciprocal (51 ULP, plenty under the 2e-2
        # budget; denominators are benign positive sums) and one multiply.
        state = {}

        def part1():
            rdb = small.tile([128, 512], F32, tag="rdb", name="rdb_t")
            nc.vector.reciprocal_approx_fast(out=rdb[:], in_=den_[:])
            state["rdb"] = rdb

        def part2():
            nc.vector.tensor_mul(
                out=otn_sb[b_][quad_][:, 512 * qi_ : 512 * (qi_ + 1)],
                in0=ot_[:],
                in1=state["rdb"][:],
            )

        return [part1, part2]

    # ---------- stage B: 64 strips, lag-2 software pipeline ----------
    def emit_pvden(args):
        ot_, den_, b_, quad_, qi_, kt_, se_, first, last = args
        for h2 in range(4):
            nc.tensor.matmul(
                ot_[32 * h2 : 32 * (h2 + 1), :],
                lhsT=v_sb[b_][kt_][:, 33 * (4 * quad_ + h2) : 33 * (4 * quad_ + h2) + 32],
                rhs=se_[:, 512 * h2 : 512 * (h2 + 1)],
                start=first,
                stop=last,
                tile_position=(0, 32 * h2),
                skip_group_check=True,
            )
        for h2 in range(4):
            nc.tensor.matmul(
                den_[32 * h2 : 32 * (h2 + 1), :],
                lhsT=ones32_sb[:],
                rhs=se_[:, 512 * h2 : 512 * (h2 + 1)],
                start=first,
                stop=last,
                tile_position=(0, 32 * h2),
                skip_group_check=True,
            )
        if last:
            norm_parts.extend(_make_norm(ot_, den_, b_, quad_, qi_))

    BLOCKS = [(0, 0), (0, 1), (1, 0), (1, 1)]  # (quad, b)
    pending = deque()
    norm_parts = deque()
    block_acc = {}  # (qi,) accumulators for the current block

    for s in range(64):
        quad, b = BLOCKS[s // 16]
        qi = (s // 8) % 2
        kt = s % 8
        # norm part for a completed qi block (emitted BEFORE this strip's
        # lag-2 PVden so the den bank WAR resolves without a stall)
        if norm_parts:
            norm_parts.popleft()()
        if kt == 0:
            block_acc = (
                ps_ot.tile([128, 512], F32, tag="ot", name="ot_ps"),
                ps_den.tile([128, 512], F32, tag="den", name="den_ps"),
            )
        ot_cur, den_cur = block_acc
        st_lo = ps_lo.tile([128, 1024], F32, tag="stlo", name="stlo_ps")
        st_hi = ps_hi.tile([128, 1024], F32, tag="sthi", name="sthi_ps")
        se = stexp_pool.tile([128, 2048], BF16, tag="se", name="se_t")
        for h2 in range(4):
            dst = st_lo if h2 < 2 else st_hi
            nc.tensor.matmul(
                dst[:, 512 * (h2 % 2) : 512 * (h2 % 2 + 1)],
                lhsT=kT_sb[b][quad][32 * h2 : 32 * (h2 + 1), 128 * kt : 128 * (kt + 1)],
                rhs=qT_sb[b][quad][32 * h2 : 32 * (h2 + 1), 512 * qi : 512 * (qi + 1)],
                start=True,
                stop=True,
                tile_position=(32 * h2, 0),
            )
        nc.scalar.activation(out=se[:, 0:1024], in_=st_lo[:], func=AF.Exp)
        nc.scalar.activation(out=se[:, 1024:2048], in_=st_hi[:], func=AF.Exp)
        off = 896 - 128 * kt + 512 * qi
        nc.vector.tensor_mul(
            out=se[:].rearrange("p (h q) -> p h q", h=4),
            in0=se[:].rearrange("p (h q) -> p h q", h=4),
            in1=eb3[:, 4 * quad : 4 * quad + 4, off : off + 512],
        )
        pending.append((ot_cur, den_cur, b, quad, qi, kt, se, kt == 0, kt == 7))
        if len(pending) > 2:
            emit_pvden(pending.popleft())
        if s in c_drips:
            with lowprio(150):
                c_drips[s]()

    # ---------- tail ----------
    while pending:
        emit_pvden(pending.popleft())
    while norm_parts:
        norm_parts.popleft()()
    # the two final output-projection groups on separate banks so they run
    # concurrently
    stage_c_group(1, 0, 1, drip_tile())
    stage_c_group(1, 1, 1, ps_ot.tile([128, 512], F32, tag="ot", name="ot_ps"))


def build():
    nc = bacc.Bacc("TRN2", target_bir_lowering=False, debug=False, num_devices=N_CORES)
    io = {
        "x": nc.dram_tensor("x", [B_LOC, C, N], BF16, kind="ExternalInput").ap(),
        "wqT": nc.dram_tensor("wqT", [C, C], BF16, kind="ExternalInput").ap(),
        "wkT": nc.dram_tensor("wkT", [C, C], BF16, kind="ExternalInput").ap(),
        "wvT": nc.dram_tensor("wvT", [C, C], BF16, kind="ExternalInput").ap(),
        "woT": nc.dram_tensor("woT", [C, C], BF16, kind="ExternalInput").ap(),
        "bo": nc.dram_tensor("bo", [C, 1], F32, kind="ExternalInput").ap(),
        "eb": nc.dram_tensor("eb", [HEADS, 128, GW], BF16, kind="ExternalInput").ap(),
        "out": nc.dram_tensor("out", [B_LOC, C, N], F32, kind="ExternalOutput").ap(),
    }
    with tile.TileContext(nc) as tc:
        _emit(tc, io)
    nc.compile()
    return nc


_CACHE: dict = {}


def _get_nc():
    if "nc" not in _CACHE:
        _CACHE["nc"] = build()
    return _CACHE["nc"]


def make_in_maps(x, Wq, Wk, Wv, Wo, bo, rel_bias, rel_idx=None):
    """Host-side sharding/layout prep. Returns per-core input maps."""
    import ml_dtypes

    bf16 = ml_dtypes.bfloat16
    x = np.asarray(x, np.float32)
    b, c, H, W = x.shape
    assert (b, c, H * W) == (B_LOC * N_CORES, C, N)
    xr = np.ascontiguousarray(x.reshape(b, c, N).astype(bf16))
    wqT = np.ascontiguousarray(np.asarray(Wq, np.float32).T.astype(bf16))
    wkT = np.ascontiguousarray((np.asarray(Wk, np.float32) * SCALE).T.astype(bf16))
    wvT = np.ascontiguousarray(np.asarray(Wv, np.float32).T.astype(bf16))
    woT = np.ascontiguousarray(np.asarray(Wo, np.float32).T.astype(bf16))
    bo2 = np.ascontiguousarray(np.asarray(bo, np.float32).reshape(C, 1))
    rb = np.asarray(rel_bias, np.float32)
    idx = G0 + np.arange(128)[:, None] - np.arange(GW)[None, :]
    ebmat = np.ascontiguousarray(np.exp(rb[:, idx]).astype(bf16))  # [8, 128, GW]
    shared = dict(wqT=wqT, wkT=wkT, wvT=wvT, woT=woT, bo=bo2, eb=ebmat)
    return [
        dict(x=np.ascontiguousarray(xr[B_LOC * i : B_LOC * (i + 1)]), **shared)
        for i in range(N_CORES)
    ]


def _install_ntff_hook_shim():
    """bass_utils fetches the axon NTFF hook via antenv.axon_hooks, which this
    image's antenv lacks; synthesize it from trn_agent_boot's ctypes hook."""
    import sys
    import types

    try:
        from antenv.axon_hooks import get_axon_ntff_profile_hook  # noqa: F401

        return
    except ImportError:
        pass
    hook = None
    try:
        from trn_agent_boot.trn_boot import _ntff_profile_via_ctypes

        hook = _ntff_profile_via_ctypes("/opt/axon/libaxon_pjrt.so")
    except Exception:
        pass
    mod = types.ModuleType("antenv.axon_hooks")
    state = {"hook": hook}
    mod.get_axon_ntff_profile_hook = lambda: state["hook"]
    mod.set_axon_ntff_profile_hook = lambda h: state.__setitem__("hook", h)
    sys.modules["antenv.axon_hooks"] = mod


def run(inputs: dict, trace: bool = False):
    """Run on the 8 cores; returns (full_output, BassKernelResults)."""
    if trace:
        _install_ntff_hook_shim()
    in_maps = make_in_maps(**inputs)
    nc = _get_nc()
    res = bass_utils.run_bass_kernel_spmd(
        nc, in_maps, core_ids=list(range(N_CORES)), trace=trace
    )
    outs = np.stack([res.results[i]["out"] for i in range(N_CORES)])
    out = outs.reshape(B_LOC * N_CORES, C, 32, 32)
    return out, res


def kernel(**inputs) -> np.ndarray:
    out, _ = run(inputs)
    return out


# revision 15
# speedup vs baseline: 1.2580x; 1.0396x over previous
"""CoAtNet relative attention kernel for Trainium2 (Bass/Tile), 8 NeuronCores.

Problem (per full input):
  x [16, 256, 32, 32] f32; Wq/Wk/Wv [256, 256]; Wo [256, 256]; bo [256];
  rel_bias [8, 3969]; rel_idx [1024, 1024] int32 (static pattern).
  out[b] = softmax(q k^T / sqrt(d) + bias) v  projected back, heads=8, d=32.

Sharding: data-parallel over batch — each of the 8 cores handles 2 batches
with identical programs (SPMD, no collectives).

Key structural facts used:
  * rel_idx[p, q] == (q - p) + 1056 exactly (the reference's quirky *W stride
    collapses the 2D relative index to 1D Toeplitz).  So the [1024, 1024]
    bias matrix per head is bias[p, q] = rel_bias[h, q - p + 1056] and any
    [128, width] tile of it (keys on partitions) is a contiguous slice of a
    small "sheared" tile  G[h, i, j'] = rel_bias[h, 1952 + i - j']  of shape
    [128, 1920].  No gather on device at all.  The bias is applied as
    exp(S+B) = exp(S) * exp(B) with exp(B) precomputed, so the application
    is a bf16 2x-mode multiply instead of an fp32 1x add.
  * Everything is computed in "transposed" layout so no transposes are ever
    needed: x arrives as [c, n] per batch; Q^T/K^T = W @ x are [d_all, n];
    scores are built as S^T [keys, queries]; P@V uses lhsT = V directly;
    and the final projection produces out^T [c, n], exactly the output
    memory layout.
  * The kernel is ACT(exp)-throughput-bound: 16.8M exps/core at 1 elem/
    lane/cycle @1.2GHz is a ~110us floor.  The schedule is built so the
    ScalarE never waits: each strip's exp is SPLIT into two ACTIVATEs over
    separate PSUM tiles (st_lo banks 0-1, st_hi banks 2-3) so the next
    strip's score matmuls can overwrite the low banks while ACT still
    processes the high banks (the single-buffered [128,2048] fp32 score
    tile cannot be double-buffered: TRN2 matmuls write fp32-only and PSUM
    has just 8 banks).  PV/den matmuls run TWO strips behind the score
    matmuls so a DVE-gated PV never head-of-line blocks ready ST work in
    the PE FIFO.  All projection / output-projection / softmax-
    normalization work is dripped into per-strip slack slots on the other
    engines.
"""

import numpy as np
from collections import deque
from contextlib import ExitStack

import concourse.bass as bass
import concourse.bacc as bacc
import concourse.mybir as mybir
import concourse.tile as tile
from concourse import bass_utils
from concourse._compat import with_exitstack

HEADS = 8
D = 32  # head dim
C = 256  # channels = heads * D
N = 1024  # tokens = 32 * 32
B_LOC = 2  # batches per core
N_CORES = 8
SCALE = D ** -0.5
GW = 1920  # sheared bias tile width
G0 = 1952  # G[h, i, j'] = rel_bias[h, G0 + i - j']

F32 = mybir.dt.float32
BF16 = mybir.dt.bfloat16
AF = mybir.ActivationFunctionType


@with_exitstack
def _emit(ctx: ExitStack, tc: tile.TileContext, io: dict):
    nc = tc.nc
    x, wqT, wkT, wvT, woT, bo, eb, out = (
        io[k] for k in ("x", "wqT", "wkT", "wvT", "woT", "bo", "eb", "out")
    )

    persist = ctx.enter_context(tc.tile_pool(name="persist", bufs=1))
    stexp_pool = ctx.enter_context(tc.tile_pool(name="stexp", bufs=4))
    small = ctx.enter_context(tc.tile_pool(name="small", bufs=2))
    outp = ctx.enter_context(tc.tile_pool(name="outp", bufs=4))
    # PSUM budget (8 banks): st_lo 2 + st_hi 2 + ot 2x1 + den 1x1 + drip 1.
    ps_lo = ctx.enter_context(tc.tile_pool(name="ps_lo", bufs=1, space="PSUM"))
    ps_hi = ctx.enter_context(tc.tile_pool(name="ps_hi", bufs=1, space="PSUM"))
    ps_ot = ctx.enter_context(tc.tile_pool(name="ps_ot", bufs=2, space="PSUM"))
    ps_den = ctx.enter_context(tc.tile_pool(name="ps_den", bufs=1, space="PSUM"))
    ps_drip = ctx.enter_context(tc.tile_pool(name="ps_drip", bufs=1, space="PSUM"))

    # ---------- DMAs: everything in flight up front ----------
    # x[b0] + projection weights first (they gate the first score matmuls);
    # the bulky 3.75MB of exp-bias tiles follow (first needed only by the
    # first bias multiply, ~2us after the first exp).
    ones32_sb = persist.tile([128, 32], BF16, tag="ones32", name="ones32")
    nc.vector.memset(ones32_sb[:], 1.0)
    # warm up the exp table set (~2.7us ACT_TABLE_LOAD) under the prologue
    warm = small.tile([1, 32], F32, tag="warm", name="warm_t")
    nc.scalar.activation(out=warm[:], in_=ones32_sb[0:1, :], func=AF.Exp)
    x_sb = [[persist.tile([128, N], BF16, tag=f"x{b}_{cc}", name=f"x{b}_{cc}") for cc in range(2)] for b in range(B_LOC)]
    for cc in range(2):
        nc.sync.dma_start(out=x_sb[0][cc][:], in_=x[0, 128 * cc : 128 * (cc + 1), :])
    wq_sb, wk_sb, wv_sb, wo_sb = [], [], [], []
    for cc in range(2):
        for lst, src, nm in (
            (wq_sb, wqT, "wq"),
            (wk_sb, wkT, "wk"),
            (wv_sb, wvT, "wv"),
            (wo_sb, woT, "wo"),
        ):
            t = persist.tile([128, C], BF16, tag=f"{nm}{cc}", name=f"{nm}{cc}")
            nc.sync.dma_start(out=t[:], in_=src[128 * cc : 128 * (cc + 1), :])
            lst.append(t)
    bo_sb = []
    for cc in range(2):
        t = persist.tile([128, 1], F32, tag=f"bo{cc}", name=f"bo{cc}")
        nc.sync.dma_start(out=t[:], in_=bo[128 * cc : 128 * (cc + 1), :])
        bo_sb.append(t)
    eb_sb = persist.tile([128, HEADS * GW], BF16, tag="eb", name="eb_sb")
    for h in range(HEADS):
        nc.sync.dma_start(out=eb_sb[:, GW * h : GW * (h + 1)], in_=eb[h])
    for cc in range(2):
        nc.sync.dma_start(out=x_sb[1][cc][:], in_=x[1, 128 * cc : 128 * (cc + 1), :])

    # ---------- persistent stage-A outputs ----------
    qT_sb = [[persist.tile([128, N], BF16, tag=f"qT{b}_{oc}", name=f"qT{b}_{oc}") for oc in range(2)] for b in range(B_LOC)]
    kT_sb = [[persist.tile([128, N], BF16, tag=f"kT{b}_{oc}", name=f"kT{b}_{oc}") for oc in range(2)] for b in range(B_LOC)]
    # v: [n, o] layout, 8 row tiles of 128 tokens, ones column per head
    # (33 cols/head) so P@V emits the softmax denominator via ones32 matmuls.
    v_sb = [[persist.tile([128, 33 * HEADS], BF16, tag=f"v{b}_{nt}", name=f"v{b}_{nt}") for nt in range(8)] for b in range(B_LOC)]
    otn_sb = [[persist.tile([128, N], BF16, tag=f"otn{b}_{ch}", name=f"otn{b}_{ch}") for ch in range(2)] for b in range(B_LOC)]

    def emit_qk_group(b, oc, nc2, w_sb, dst, pool_tile):
        for cc in range(2):
            nc.tensor.matmul(
                pool_tile[:, 0:512],
                lhsT=w_sb[cc][:, 128 * oc : 128 * (oc + 1)],
                rhs=x_sb[b][cc][:, 512 * nc2 : 512 * (nc2 + 1)],
                start=(cc == 0),
                stop=(cc == 1),
            )
        nc.vector.tensor_copy(
            out=dst[b][oc][:, 512 * nc2 : 512 * (nc2 + 1)], in_=pool_tile[:, 0:512]
        )

    def emit_v_group(b, nt, pool_tile):
        for cc in range(2):
            nc.tensor.matmul(
                pool_tile[:, 0:C],
                lhsT=x_sb[b][cc][:, 128 * nt : 128 * (nt + 1)],
                rhs=wv_sb[cc][:],
                start=(cc == 0),
                stop=(cc == 1),
            )
        v33 = v_sb[b][nt][:].rearrange("p (h w) -> p h w", w=33)
        nc.vector.tensor_copy(
            out=v33[:, :, 0:32], in_=pool_tile[:, 0:C].rearrange("p (h w) -> p h w", w=32)
        )
        nc.vector.memset(v33[:, :, 32:33], 1.0)

    def stage_c_group(b, ct, q2, pool_tile):
        for ch in range(2):
            nc.tensor.matmul(
                pool_tile[:, 0:512],
                lhsT=wo_sb[ch][:, 128 * ct : 128 * (ct + 1)],
                rhs=otn_sb[b][ch][:, 512 * q2 : 512 * (q2 + 1)],
                start=(ch == 0),
                stop=(ch == 1),
            )
        ob = outp.tile([128, 512], F32, tag="ob", name="ob_t")
        nc.vector.tensor_scalar_add(out=ob[:], in0=pool_tile[:, 0:512], scalar1=bo_sb[ct][:])
        nc.sync.dma_start(
            out=out[b, 128 * ct : 128 * (ct + 1), 512 * q2 : 512 * (q2 + 1)],
            in_=ob[:],
        )

    # ---------- prologue stage A ----------
    # b0's projections run as a dense PE burst at normal priority (they gate
    # the first strips and the burst warms the HAM).  b1's projections and V
    # tiles are emitted at LOW scheduler priority: the Tile scheduler slots
    # them into PE idle slivers during the early strips instead of ahead of
    # the critical score matmuls.
    from contextlib import contextmanager

    @contextmanager
    def lowprio(off):
        tc.cur_priority += off
        try:
            yield
        finally:
            tc.cur_priority -= off

    def drip_tile():
        return ps_drip.tile([128, 512], F32, tag="drip", name="drip_ps")

    pi = 0

    def prol_tile():
        nonlocal pi
        pi += 1
        if pi % 3 == 0:
            return drip_tile()
        return ps_ot.tile([128, 512], F32, tag="ot", name="ot_ps")

    # phase 1 (dense prologue): b0/quad0 q,k at full priority — they gate
    # strip 0 — then b0's V at slightly lower priority (PV consumes v[kt]
    # only from strip 2 onward).
    for nc2 in range(2):
        for w_sb, dst in ((wq_sb, qT_sb), (wk_sb, kT_sb)):
            emit_qk_group(0, 0, nc2, w_sb, dst, prol_tile())
    with lowprio(30):
        for nt in range(8):
            emit_v_group(0, nt, prol_tile())

    # Everything else is dripped INSIDE the strip loop ~10 strips before its
    # deadline (emission position = scheduler priority), one group per drip
    # strip, all through the dedicated drip bank:
    #   qk(b1,oc0) by strip 16; v(b1,nt) by strip 18+nt; qk(b0,oc1) by
    #   strip 32/40; qk(b1,oc1) by strip 48/56; stage C as the otn halves
    #   complete (quad1 norms land ~4 strips into the following qi block).
    drips = {}
    qk_sched = [
        (0, 1, 0, 0), (2, 1, 0, 0, ), (4, 1, 0, 1), (6, 1, 0, 1),
        (23, 0, 1, 0), (25, 0, 1, 0), (27, 0, 1, 1), (29, 0, 1, 1),
        (33, 1, 1, 0), (35, 1, 1, 0), (37, 1, 1, 1), (39, 1, 1, 1),
    ]
    qk_which = {}
    for s, b, oc, nc2 in qk_sched:
        key = (b, oc, nc2)
        w_sb, dst = ((wq_sb, qT_sb), (wk_sb, kT_sb))[qk_which.get(key, 0)]
        qk_which[key] = 1
        drips[s] = (lambda b=b, oc=oc, nc2=nc2, w_sb=w_sb, dst=dst:
                    emit_qk_group(b, oc, nc2, w_sb, dst, drip_tile()))
    for i, s in enumerate((7, 9, 11, 13, 15, 17, 19, 21)):
        drips[s] = (lambda nt=i: emit_v_group(1, nt, drip_tile()))
    for i, (ct,) in enumerate([(0,), (1,)]):
        drips[45 + i] = lambda ct=ct: stage_c_group(0, ct, 0, drip_tile())
        drips[53 + i] = lambda ct=ct: stage_c_group(0, ct, 1, drip_tile())
        drips[60 + i] = lambda ct=ct: stage_c_group(1, ct, 0, drip_tile())

    # ---------- softmax normalization (per qi block) ----------
    eb3 = eb_sb[:].rearrange("p (h w) -> p h w", w=GW)

    def _make_norm(ot_, den_, b_, quad_, qi_):
        # The den accumulator's rows 32h2..32h2+31 hold 32 identical copies
        # of head h2's denominators (M=32 col-tiled ones matmul), i.e. the
        # tile is ALREADY in row-broadcast layout.  The whole normalization
        # is two DVE ops: approx-reciprocal (51 ULP, plenty under the 2e-2
        # budget; denominators are benign positive sums) and one multiply.
        state = {}

        def part1():
            rdb = small.tile([128, 512], F32, tag="rdb", name="rdb_t")
            nc.vector.reciprocal_approx_fast(out=rdb[:], in_=den_[:])
            state["rdb"] = rdb

        def part2():
            nc.vector.tensor_mul(
                out=otn_sb[b_][quad_][:, 512 * qi_ : 512 * (qi_ + 1)],
                in0=ot_[:],
                in1=state["rdb"][:],
            )

        return [part1, part2]

    # ---------- stage B: 64 strips, lag-2 software pipeline ----------
    def emit_pvden(args):
        ot_, den_, b_, quad_, qi_, kt_, se_, first, last = args
        for h2 in range(4):
            nc.tensor.matmul(
                ot_[32 * h2 : 32 * (h2 + 1), :],
                lhsT=v_sb[b_][kt_][:, 33 * (4 * quad_ + h2) : 33 * (4 * quad_ + h2) + 32],
                rhs=se_[:, 512 * h2 : 512 * (h2 + 1)],
                start=first,
                stop=last,
                tile_position=(0, 32 * h2),
                skip_group_check=True,
            )
        for h2 in range(4):
            nc.tensor.matmul(
                den_[32 * h2 : 32 * (h2 + 1), :],
                lhsT=ones32_sb[:],
                rhs=se_[:, 512 * h2 : 512 * (h2 + 1)],
                start=first,
                stop=last,
                tile_position=(0, 32 * h2),
                skip_group_check=True,
            )
        if last:
            norm_parts.extend(_make_norm(ot_, den_, b_, quad_, qi_))

    BLOCKS = [(0, 0), (0, 1), (1, 0), (1, 1)]  # (quad, b)
    pending = deque()
    norm_parts = deque()
    block_acc = {}  # (qi,) accumulators for the current block

    for s in range(64):
        quad, b = BLOCKS[s // 16]
        qi = (s // 8) % 2
        kt = s % 8
        # norm part for a completed qi block (emitted BEFORE this strip's
        # lag-2 PVden so the den bank WAR resolves without a stall)
        if norm_parts:
            norm_parts.popleft()()
        if kt == 0:
            block_acc = (
                ps_ot.tile([128, 512], F32, tag="ot", name="ot_ps"),
                ps_den.tile([128, 512], F32, tag="den", name="den_ps"),
            )
        ot_cur, den_cur = block_acc
        st_lo = ps_lo.tile([128, 1024], F32, tag="stlo", name="stlo_ps")
        st_hi = ps_hi.tile([128, 1024], F32, tag="sthi", name="sthi_ps")
        se = stexp_pool.tile([128, 2048], BF16, tag="se", name="se_t")
        for h2 in range(4):
            dst = st_lo if h2 < 2 else st_hi
            nc.tensor.matmul(
                dst[:, 512 * (h2 % 2) : 512 * (h2 % 2 + 1)],
                lhsT=kT_sb[b][quad][32 * h2 : 32 * (h2 + 1), 128 * kt : 128 * (kt + 1)],
                rhs=qT_sb[b][quad][32 * h2 : 32 * (h2 + 1), 512 * qi : 512 * (qi + 1)],
                start=True,
                stop=True,
                tile_position=(32 * h2, 0),
            )
        nc.scalar.activation(out=se[:, 0:1024], in_=st_lo[:], func=AF.Exp)
        nc.scalar.activation(out=se[:, 1024:2048], in_=st_hi[:], func=AF.Exp)
        off = 896 - 128 * kt + 512 * qi
        nc.vector.tensor_mul(
            out=se[:].rearrange("p (h q) -> p h q", h=4),
            in0=se[:].rearrange("p (h q) -> p h q", h=4),
            in1=eb3[:, 4 * quad : 4 * quad + 4, off : off + 512],
        )
        pending.append((ot_cur, den_cur, b, quad, qi, kt, se, kt == 0, kt == 7))
        if len(pending) > 2:
            emit_pvden(pending.popleft())
        if s in drips:
            drips[s]()

    # ---------- tail ----------
    while pending:
        emit_pvden(pending.popleft())
    while norm_parts:
        norm_parts.popleft()()
    # the two final output-projection groups on separate banks so they run
    # concurrently
    stage_c_group(1, 0, 1, drip_tile())
    stage_c_group(1, 1, 1, ps_ot.tile([128, 512], F32, tag="ot", name="ot_ps"))


def build():
    nc = bacc.Bacc("TRN2", target_bir_lowering=False, debug=False, num_devices=N_CORES)
    io = {
        "x": nc.dram_tensor("x", [B_LOC, C, N], BF16, kind="ExternalInput").ap(),
        "wqT": nc.dram_tensor("wqT", [C, C], BF16, kind="ExternalInput").ap(),
        "wkT": nc.dram_tensor("wkT", [C, C], BF16, kind="ExternalInput").ap(),
        "wvT": nc.dram_tensor("wvT", [C, C], BF16, kind="ExternalInput").ap(),
        "woT": nc.dram_tensor("woT", [C, C], BF16, kind="ExternalInput").ap(),
        "bo": nc.dram_tensor("bo", [C, 1], F32, kind="ExternalInput").ap(),
        "eb": nc.dram_tensor("eb", [HEADS, 128, GW], BF16, kind="ExternalInput").ap(),
        "out": nc.dram_tensor("out", [B_LOC, C, N], F32, kind="ExternalOutput").ap(),
    }
    with tile.TileContext(nc) as tc:
        _emit(tc, io)
    nc.compile()
    return nc


_CACHE: dict = {}


def _get_nc():
    if "nc" not in _CACHE:
        _CACHE["nc"] = build()
    return _CACHE["nc"]


def make_in_maps(x, Wq, Wk, Wv, Wo, bo, rel_bias, rel_idx=None):
    """Host-side sharding/layout prep. Returns per-core input maps."""
    import ml_dtypes

    bf16 = ml_dtypes.bfloat16
    x = np.asarray(x, np.float32)
    b, c, H, W = x.shape
    assert (b, c, H * W) == (B_LOC * N_CORES, C, N)
    xr = np.ascontiguousarray(x.reshape(b, c, N).astype(bf16))
    wqT = np.ascontiguousarray(np.asarray(Wq, np.float32).T.astype(bf16))
    wkT = np.ascontiguousarray((np.asarray(Wk, np.float32) * SCALE).T.astype(bf16))
    wvT = np.ascontiguousarray(np.asarray(Wv, np.float32).T.astype(bf16))
    woT = np.ascontiguousarray(np.asarray(Wo, np.float32).T.astype(bf16))
    bo2 = np.ascontiguousarray(np.asarray(bo, np.float32).reshape(C, 1))
    rb = np.asarray(rel_bias, np.float32)
    idx = G0 + np.arange(128)[:, None] - np.arange(GW)[None, :]
    ebmat = np.ascontiguousarray(np.exp(rb[:, idx]).astype(bf16))  # [8, 128, GW]
    shared = dict(wqT=wqT, wkT=wkT, wvT=wvT, woT=woT, bo=bo2, eb=ebmat)
    return [
        dict(x=np.ascontiguousarray(xr[B_LOC * i : B_LOC * (i + 1)]), **shared)
        for i in range(N_CORES)
    ]


def _install_ntff_hook_shim():
    """bass_utils fetches the axon NTFF hook via antenv.axon_hooks, which this
    image's antenv lacks; synthesize it from trn_agent_boot's ctypes hook."""
    import sys
    import types

    try:
        from antenv.axon_hooks import get_axon_ntff_profile_hook  # noqa: F401

        return
    except ImportError:
        pass
    hook = None
    try:
        from trn_agent_boot.trn_boot import _ntff_profile_via_ctypes

        hook = _ntff_profile_via_ctypes("/opt/axon/libaxon_pjrt.so")
    except Exception:
        pass
    mod = types.ModuleType("antenv.axon_hooks")
    state = {"hook": hook}
    mod.get_axon_ntff_profile_hook = lambda: state["hook"]
    mod.set_axon_ntff_profile_hook = lambda h: state.__setitem__("hook", h)
    sys.modules["antenv.axon_hooks"] = mod


def run(inputs: dict, trace: bool = False):
    """Run on the 8 cores; returns (full_output, BassKernelResults)."""
    if trace:
        _install_ntff_hook_shim()
    in_maps = make_in_maps(**inputs)
    nc = _get_nc()
    res = bass_utils.run_bass_kernel_spmd(
        nc, in_maps, core_ids=list(range(N_CORES)), trace=trace
    )
    outs = np.stack([res.results[i]["out"] for i in range(N_CORES)])
    out = outs.reshape(B_LOC * N_CORES, C, 32, 32)
    return out, res


def kernel(**inputs) -> np.ndarray:
    out, _ = run(inputs)
    return out
